# revision 5
# baseline (speedup 1.0000x reference)
"""LurieNet-k Trainium2 kernel.

Computes, from the raw parametrization tensors, the matrices
  C = UC @ SC @ VC^T,  B = UB @ SB @ VB^T,
  A = 0.5*UA @ SA @ UA^T + 0.5*YA  (SA = -(alpha_upp*I + GA))
entirely on device (matrix exponentials of skew matrices via
scaling-and-squaring Taylor), then runs the 511-step recurrence
  y  = C x + by
  x' = x + 0.01*(A x + B tanh(y) + bx)
on a (128, 64) state shard per NeuronCore (batch data-parallel over the
8 cores), writing the full (b, t, n) trajectory.

Everything is fp32: the system amplifies per-step state perturbations
(bf16 matmuls diverge to ~0.36 rel err), so reduced precision is not an
option for the recurrence.
"""

import sys

for _p in ("/opt/trn_rl_repo",):
    if _p not in sys.path:
        sys.path.insert(0, _p)

import numpy as np

import concourse.bass as bass
import concourse.mybir as mybir
import concourse.tile as tile
from concourse import bacc
from concourse import bass_isa
from concourse.bass import ds
from concourse.bass_utils import run_bass_kernel_spmd
from concourse.masks import make_identity, make_upper_triangular

F32 = mybir.dt.float32
ALU = mybir.AluOpType
ACTF = mybir.ActivationFunctionType
AXIS = mybir.AxisListType

N = 128          # state dim
TMAX = 512       # time steps (including t=0)
BS = 512         # global batch
NCORES = 8
BSH = BS // NCORES   # 64 batch columns per core
STEP = 0.01
KTOP = 4

EXPM_SCAL = 6    # expm scaling: X = S / 2**EXPM_SCAL, then 6 squarings
EXPM_TERMS = 7   # Taylor terms in the Horner evaluation

PARAM_NAMES = [
    "ZA_Y", "ZA_U", "ZA_G", "ZB_U", "ZB_V", "ZB_S", "ZC_U", "ZC_V", "ZC_S",
]


def build_program(tmax=TMAX, tc_chunk=32, groups=2):
    """Build the single-NeuronCore Bass program (run SPMD on all 8 cores)."""
    assert tmax % tc_chunk == 0 and tc_chunk % 2 == 0
    assert BSH % groups == 0
    gcols = BSH // groups
    half = tc_chunk // 2
    nchunks = tmax // tc_chunk

    nc = bacc.Bacc(
        "TRN2",
        target_bir_lowering=False,
        debug=False,
        enable_asserts=False,
        num_devices=NCORES,
    )

    x0 = nc.dram_tensor("x0", [N, BSH], F32, kind="ExternalInput")
    zs = {
        name: nc.dram_tensor(name, [N, N], F32, kind="ExternalInput")
        for name in PARAM_NAMES
    }
    bx_d = nc.dram_tensor("bx", [N, 1], F32, kind="ExternalInput")
    by_d = nc.dram_tensor("by", [N, 1], F32, kind="ExternalInput")
    out = nc.dram_tensor("out", [BSH, tmax, N], F32, kind="ExternalOutput")

    with tile.TileContext(nc) as tc:
        with tc.tile_pool(name="const", bufs=1) as constp:
            ident = constp.tile([N, N], F32, tag="ident")
            make_identity(nc, ident[:])
            masku = constp.tile([N, N], F32, tag="masku")
            make_upper_triangular(nc, masku[:], val=1.0, diag=False)

            by_c = constp.tile([N, 1], F32, tag="by")
            nc.sync.dma_start(out=by_c[:], in_=by_d[:])
            bx_c = constp.tile([N, 1], F32, tag="bxraw")
            nc.sync.dma_start(out=bx_c[:], in_=bx_d[:])
            bxp_c = constp.tile([N, 1], F32, tag="bxp")
            nc.vector.tensor_scalar_mul(bxp_c[:], bx_c[:], STEP)

            # ------- setup phase: expm's + weight assembly -------
            CTm = constp.tile([N, N], F32, tag="CTm")    # C^T
            ApTm = constp.tile([N, N], F32, tag="ApTm")  # (I + 0.01 A)^T
            BpTm = constp.tile([N, N], F32, tag="BpTm")  # (0.01 B)^T

            with (
                tc.tile_pool(name="zbuf", bufs=1) as zp,
                tc.tile_pool(name="work", bufs=2) as wp,
                tc.tile_pool(name="eres", bufs=1) as ep,
                tc.tile_pool(name="small", bufs=1) as sp,
                tc.tile_pool(name="pss", bufs=2, space="PSUM") as psp,
            ):
                zt = {}
                for name in PARAM_NAMES:
                    zt[name] = zp.tile([N, N], F32, tag=name, name=f"z_{name}")
                    nc.sync.dma_start(out=zt[name][:], in_=zs[name][:])

                def expm_transposed(z_tile, tag):
                    """Return expm(skew(Z))^T for the strict-upper skew of Z.

                    Maintains the (T, T^T) pair through Horner + squaring so
                    no PE transposes are needed: with negX = X^T = -X,
                      X @ T     = matmul(lhsT=negX, rhs=T)
                      T^T @ X^T = matmul(lhsT=T,    rhs=negX)
                    """
                    scal = 1.0 / (2.0 ** EXPM_SCAL)
                    us = wp.tile([N, N], F32, tag="us")
                    # us = (Z * scal) * mask  (strict upper part, prescaled)
                    nc.vector.scalar_tensor_tensor(
                        us[:], z_tile[:], scal, masku[:], op0=ALU.mult, op1=ALU.mult
                    )
                    pst = psp.tile([N, N], F32, tag="psA")
                    nc.tensor.transpose(pst[:], us[:], ident[:])
                    negx = wp.tile([N, N], F32, tag="negx")
                    # negX = us^T - us  ( = X^T = -X for X = us - us^T )
                    nc.vector.scalar_tensor_tensor(
                        negx[:], pst[:], 1.0, us[:], op0=ALU.mult, op1=ALU.subtract
                    )

                    t_cur, tt_cur = ident, ident
                    for j in range(EXPM_TERMS, 0, -1):
                        psa = psp.tile([N, N], F32, tag="psA")
                        psb = psp.tile([N, N], F32, tag="psB")
                        nc.tensor.matmul(
                            psa[:], negx[:], t_cur[:], start=True, stop=True
                        )
                        nc.tensor.matmul(
                            psb[:], t_cur[:], negx[:], start=True, stop=True
                        )
                        t_new = wp.tile([N, N], F32, tag="T")
                        tt_new = wp.tile([N, N], F32, tag="TT")
                        nc.vector.scalar_tensor_tensor(
                            t_new[:], psa[:], 1.0 / j, ident[:],
                            op0=ALU.mult, op1=ALU.add,
                        )
                        nc.vector.scalar_tensor_tensor(
                            tt_new[:], psb[:], 1.0 / j, ident[:],
                            op0=ALU.mult, op1=ALU.add,
                        )
                        t_cur, tt_cur = t_new, tt_new
                    for _ in range(EXPM_SCAL):
                        psa = psp.tile([N, N], F32, tag="psA")
                        psb = psp.tile([N, N], F32, tag="psB")
                        nc.tensor.matmul(
                            psa[:], tt_cur[:], t_cur[:], start=True, stop=True
                        )
                        nc.tensor.matmul(
                            psb[:], t_cur[:], tt_cur[:], start=True, stop=True
                        )
                        t_new = wp.tile([N, N], F32, tag="T")
                        tt_new = wp.tile([N, N], F32, tag="TT")
                        nc.vector.tensor_copy(t_new[:], psa[:])
                        nc.vector.tensor_copy(tt_new[:], psb[:])
                        t_cur, tt_cur = t_new, tt_new
                    res = ep.tile([N, N], F32, tag=tag)
                    nc.vector.tensor_copy(res[:], tt_cur[:])
                    return res

                uct = expm_transposed(zt["ZC_U"], "UCT")   # UC^T
                vct = expm_transposed(zt["ZC_V"], "VCT")   # VC^T
                ubt = expm_transposed(zt["ZB_U"], "UBT")   # UB^T
                vbt = expm_transposed(zt["ZB_V"], "VBT")   # VB^T
                uat = expm_transposed(zt["ZA_U"], "UAT")   # UA^T

                def absdiag_col(z_tile, tag):
                    tmp = wp.tile([N, N], F32, tag="us")
                    nc.vector.tensor_mul(tmp[:], z_tile[:], ident[:])
                    col = sp.tile([N, 1], F32, tag=tag)
                    nc.vector.tensor_reduce(
                        col[:], tmp[:], AXIS.X, ALU.add, apply_absolute_value=True
                    )
                    return col

                dc_col = absdiag_col(zt["ZC_S"], "dc")   # |diag(ZC_S)|
                db_col = absdiag_col(zt["ZB_S"], "db")   # |diag(ZB_S)|
                ga_col = absdiag_col(zt["ZA_G"], "ga")   # |diag(ZA_G)|

                # top-4: alpha = sqrt(sum_i (b_i c_i)^2), b/c sorted desc.
                bwork = sp.tile([N, 1], F32, tag="bwork")
                cwork = sp.tile([N, 1], F32, tag="cwork")
                nc.vector.tensor_copy(bwork[:], db_col[:])
                nc.vector.tensor_copy(cwork[:], dc_col[:])
                acc = sp.tile([N, 1], F32, tag="acc")
                nc.vector.memset(acc[:], 0.0)
                bmax = sp.tile([N, 1], F32, tag="bmax")
                cmax = sp.tile([N, 1], F32, tag="cmax")
                prod = sp.tile([N, 1], F32, tag="prod")
                gmask = sp.tile([N, 1], F32, tag="gmask")
                tdrop = sp.tile([N, 1], F32, tag="tdrop")
                for i in range(KTOP):
                    nc.gpsimd.partition_all_reduce(
                        bmax[:], bwork[:], N, bass_isa.ReduceOp.max
                    )
                    nc.gpsimd.partition_all_reduce(
                        cmax[:], cwork[:], N, bass_isa.ReduceOp.max
                    )
                    nc.vector.tensor_mul(prod[:], bmax[:], cmax[:])
                    nc.vector.tensor_mul(prod[:], prod[:], prod[:])
                    nc.vector.tensor_add(acc[:], acc[:], prod[:])
                    if i < KTOP - 1:
                        # zero out the extracted max (values all > 0)
                        nc.vector.tensor_single_scalar(
                            gmask[:], bwork[:], bmax[:], ALU.is_ge
                        )
                        nc.vector.tensor_mul(tdrop[:], bwork[:], gmask[:])
                        nc.vector.tensor_sub(bwork[:], bwork[:], tdrop[:])
                        nc.vector.tensor_single_scalar(
                            gmask[:], cwork[:], cmax[:], ALU.is_ge
                        )
                        nc.vector.tensor_mul(tdrop[:], cwork[:], gmask[:])
                        nc.vector.tensor_sub(cwork[:], cwork[:], tdrop[:])
                alpha = sp.tile([N, 1], F32, tag="alpha")
                nc.scalar.activation(alpha[:], acc[:], ACTF.Sqrt)

                # sa05 = -0.5*(alpha + gA)  (per-partition row scale of UA^T)
                sa05 = sp.tile([N, 1], F32, tag="sa05")
                nc.vector.tensor_scalar(
                    sa05[:], ga_col[:], alpha[:], -0.5, op0=ALU.add, op1=ALU.mult
                )
                sb01 = sp.tile([N, 1], F32, tag="sb01")
                nc.vector.tensor_scalar_mul(sb01[:], db_col[:], STEP)

                # C^T = VC @ (SC @ UC^T)
                p1 = wp.tile([N, N], F32, tag="us")
                nc.vector.tensor_scalar_mul(p1[:], uct[:], dc_col[:])
                psa = psp.tile([N, N], F32, tag="psA")
                nc.tensor.matmul(psa[:], vct[:], p1[:], start=True, stop=True)
                nc.vector.tensor_copy(CTm[:], psa[:])

                # (0.01 B)^T = VB @ (0.01 SB @ UB^T)
                p2 = wp.tile([N, N], F32, tag="us")
                nc.vector.tensor_scalar_mul(p2[:], ubt[:], sb01[:])
                psb = psp.tile([N, N], F32, tag="psB")
                nc.tensor.matmul(psb[:], vbt[:], p2[:], start=True, stop=True)
                nc.vector.tensor_copy(BpTm[:], psb[:])

                # M = UA @ (sa05 * UA^T) = 0.5*UA SA UA^T (symmetric)
                p3 = wp.tile([N, N], F32, tag="negx")
                nc.vector.tensor_scalar_mul(p3[:], uat[:], sa05[:])
                psm = psp.tile([N, N], F32, tag="psA")
                nc.tensor.matmul(psm[:], uat[:], p3[:], start=True, stop=True)
                # IYA = I - 0.005*YA,  YA = Uy - Uy^T
                uy = wp.tile([N, N], F32, tag="us")
                nc.vector.tensor_mul(uy[:], zt["ZA_Y"][:], masku[:])
                pst2 = psp.tile([N, N], F32, tag="psB")
                nc.tensor.transpose(pst2[:], uy[:], ident[:])
                q1 = wp.tile([N, N], F32, tag="T")
                # q1 = 0.005*Uy^T + I
                nc.vector.scalar_tensor_tensor(
                    q1[:], pst2[:], 0.5 * STEP, ident[:], op0=ALU.mult, op1=ALU.add
                )
                q2 = wp.tile([N, N], F32, tag="TT")
                # q2 = q1 - 0.005*Uy  (= I - 0.005*YA)
                nc.vector.tensor_scalar_mul(uy[:], uy[:], 0.5 * STEP)
                nc.vector.tensor_sub(q2[:], q1[:], uy[:])
                # ApT = 0.01*M + IYA  (= I + 0.01*A^T)
                nc.vector.scalar_tensor_tensor(
                    ApTm[:], psm[:], STEP, q2[:], op0=ALU.mult, op1=ALU.add
                )

            # ------- recurrence -------
            with (
                tc.tile_pool(name="xbuf", bufs=2) as xbufp,
                tc.tile_pool(name="stage", bufs=2) as stagep,
                tc.tile_pool(name="th", bufs=2 * groups) as thp,
                tc.tile_pool(name="psy", bufs=2, space="PSUM") as psyp,
                tc.tile_pool(name="psx", bufs=2, space="PSUM") as psxp,
                tc.tile_pool(name="pstr", bufs=2, space="PSUM") as pstrp,
            ):
                # xbuf column slot for local step s: pairs (i, i+half) are
                # adjacent so the PE transpose reads one contiguous block
                # (walrus: matmul weight APs must have a single free dim).
                def slot(s):
                    return 2 * (s % half) + (s // half)

                xb_prev = None
                for c in range(nchunks):
                    xb = xbufp.tile([N, tc_chunk * BSH], F32, tag="xb")
                    st = stagep.tile([128, half * N], F32, tag="st")
                    if c == 0:
                        nc.sync.dma_start(out=xb[:, 0:BSH], in_=x0[:])
                    for s in range(tc_chunk):
                        t = c * tc_chunk + s
                        if t > 0:
                            if s > 0:
                                pxb, ps_ = xb, slot(s - 1)
                            else:
                                pxb, ps_ = xb_prev, slot(tc_chunk - 1)
                            for g in range(groups):
                                xprev = pxb[:, ds(ps_ * BSH + g * gcols, gcols)]
                                psy = psyp.tile([N, gcols], F32, tag="psy")
                                nc.tensor.matmul(
                                    psy[:], CTm[:], xprev, start=True, stop=True
                                )
                                psx = psxp.tile([N, gcols], F32, tag="psx")
                                nc.tensor.matmul(
                                    psx[:], ApTm[:], xprev, start=True, stop=False
                                )
                                th = thp.tile([N, gcols], F32, tag="th")
                                nc.scalar.activation(
                                    th[:], psy[:], ACTF.Tanh, bias=by_c[:], scale=1.0
                                )
                                nc.tensor.matmul(
                                    psx[:], BpTm[:], th[:], start=False, stop=True
                                )
                                nc.vector.tensor_scalar_add(
                                    xb[:, ds(slot(s) * BSH + g * gcols, gcols)],
                                    psx[:],
                                    bxp_c[:],
                                )
                        if s >= half:
                            i = s - half
                            # transpose steps (i, i+half): adjacent slots
                            # (2i, 2i+1) -> one contiguous 128-col block
                            pstr = pstrp.tile([128, N], F32, tag="pstr")
                            nc.tensor.transpose(
                                pstr[:], xb[:, ds(2 * i * BSH, 2 * BSH)], ident[:]
                            )
                            nc.vector.tensor_copy(st[:, ds(i * N, N)], pstr[:])
                    for h in range(2):
                        t0 = c * tc_chunk + h * half
                        dram_ap = out[:, t0:t0 + half, :].rearrange(
                            "b i n -> b (i n)"
                        )
                        nc.sync.dma_start(
                            out=dram_ap, in_=st[h * 64:(h + 1) * 64, :]
                        )
                    xb_prev = xb

    nc.compile()
    return nc


_CACHED = {}


def _get_program(tmax=TMAX, tc_chunk=32, groups=2):
    key = (tmax, tc_chunk, groups)
    if key not in _CACHED:
        _CACHED[key] = build_program(tmax, tc_chunk, groups)
    return _CACHED[key]


def make_in_maps(inputs, tmax=TMAX):
    X0 = np.ascontiguousarray(np.asarray(inputs["X0"], dtype=np.float32))
    base = {
        name: np.ascontiguousarray(np.asarray(inputs[name], dtype=np.float32))
        for name in PARAM_NAMES
    }
    base["bx"] = np.ascontiguousarray(
        np.asarray(inputs["bx"], dtype=np.float32).reshape(N, 1)
    )
    base["by"] = np.ascontiguousarray(
        np.asarray(inputs["by"], dtype=np.float32).reshape(N, 1)
    )
    in_maps = []
    for c in range(NCORES):
        m = dict(base)
        m["x0"] = np.ascontiguousarray(X0[c * BSH:(c + 1) * BSH].T)
        in_maps.append(m)
    return in_maps


def run_spmd(inputs, tmax=TMAX, tc_chunk=32, groups=2, trace=False, tmpdir=None):
    nc = _get_program(tmax, tc_chunk, groups)
    in_maps = make_in_maps(inputs, tmax)
    res = run_bass_kernel_spmd(
        nc, in_maps, list(range(NCORES)), trace=trace, tmpdir=tmpdir
    )
    outs = [res.results[c]["out"] for c in range(NCORES)]
    full = np.concatenate(outs, axis=0)
    return full, res


def kernel(**inputs):
    full, _ = run_spmd(inputs)
    return full


# revision 8
# speedup vs baseline: 1.3244x; 1.3244x over previous
"""LurieNet-k Trainium2 kernel.

Computes, from the raw parametrization tensors, the matrices
  C = UC @ SC @ VC^T,  B = UB @ SB @ VB^T,
  A = 0.5*UA @ SA @ UA^T + 0.5*YA  (SA = -(alpha_upp*I + GA))
entirely on device (matrix exponentials of skew matrices via
scaling-and-squaring Taylor), then runs the 511-step recurrence
  y  = C x + by
  x' = x + 0.01*(A x + B tanh(y) + bx)
on a (128, 64) state shard per NeuronCore (batch data-parallel over the
8 cores), writing the full (b, t, n) trajectory.

Precision: the system amplifies per-step state perturbations (bf16
matmuls diverge to ~0.36 rel err), so the setup runs in fp32 and the
recurrence in fp32r (single-pass TensorE fp32, ~13-bit mantissa,
measured ~3.5e-3 final rel err) to halve the LDWEIGHTS-bound PE cost.
"""

import sys

for _p in ("/opt/trn_rl_repo",):
    if _p not in sys.path:
        sys.path.insert(0, _p)

import numpy as np

import concourse.bass as bass
import concourse.mybir as mybir
import concourse.tile as tile
from concourse import bacc
from concourse import bass_isa
from concourse.bass import ds
from concourse.bass_utils import run_bass_kernel_spmd
from concourse.masks import make_identity, make_upper_triangular

F32 = mybir.dt.float32
F32R = mybir.dt.float32r
ALU = mybir.AluOpType
ACTF = mybir.ActivationFunctionType
AXIS = mybir.AxisListType

N = 128          # state dim
TMAX = 512       # time steps (including t=0)
BS = 512         # global batch
NCORES = 8
BSH = BS // NCORES   # 64 batch columns per core
STEP = 0.01
KTOP = 4

EXPM_SCAL = 6    # expm scaling: X = S / 2**EXPM_SCAL, then 6 squarings
EXPM_TERMS = 7   # Taylor terms in the Horner evaluation

PARAM_NAMES = [
    "ZA_Y", "ZA_U", "ZA_G", "ZB_U", "ZB_V", "ZB_S", "ZC_U", "ZC_V", "ZC_S",
]


def build_program(tmax=TMAX, tc_chunk=32, groups=2, rdt=F32R):
    """Build the single-NeuronCore Bass program (run SPMD on all 8 cores)."""
    assert tmax % tc_chunk == 0 and tc_chunk % 2 == 0
    assert BSH % groups == 0
    gcols = BSH // groups
    half = tc_chunk // 2
    nchunks = tmax // tc_chunk

    nc = bacc.Bacc(
        "TRN2",
        target_bir_lowering=False,
        debug=False,
        enable_asserts=False,
        num_devices=NCORES,
    )

    x0 = nc.dram_tensor("x0", [N, BSH], F32, kind="ExternalInput")
    zs = {
        name: nc.dram_tensor(name, [N, N], F32, kind="ExternalInput")
        for name in PARAM_NAMES
    }
    bx_d = nc.dram_tensor("bx", [N, 1], F32, kind="ExternalInput")
    by_d = nc.dram_tensor("by", [N, 1], F32, kind="ExternalInput")
    out = nc.dram_tensor("out", [BSH, tmax, N], F32, kind="ExternalOutput")

    with tile.TileContext(nc) as tc:
        with tc.tile_pool(name="const", bufs=1) as constp:
            ident = constp.tile([N, N], F32, tag="ident")
            make_identity(nc, ident[:])
            masku = constp.tile([N, N], F32, tag="masku")
            make_upper_triangular(nc, masku[:], val=1.0, diag=False)
            ident_r = constp.tile([N, N], rdt, tag="ident_r")
            nc.vector.tensor_copy(ident_r[:], ident[:])

            by_c = constp.tile([N, 1], F32, tag="by")
            nc.sync.dma_start(out=by_c[:], in_=by_d[:])
            bx_c = constp.tile([N, 1], F32, tag="bxraw")
            nc.sync.dma_start(out=bx_c[:], in_=bx_d[:])
            bxp_c = constp.tile([N, 1], F32, tag="bxp")
            nc.vector.tensor_scalar_mul(bxp_c[:], bx_c[:], STEP)
            x0_c = constp.tile([N, BSH], F32, tag="x0c")
            nc.sync.dma_start(out=x0_c[:], in_=x0[:])

            # ------- setup phase: expm's + weight assembly -------
            CTm = constp.tile([N, N], rdt, tag="CTm")    # C^T
            ApTm = constp.tile([N, N], rdt, tag="ApTm")  # (I + 0.01 A)^T
            BpTm = constp.tile([N, N], rdt, tag="BpTm")  # (0.01 B)^T

            with (
                tc.tile_pool(name="zbuf", bufs=1) as zp,
                tc.tile_pool(name="work", bufs=2) as wp,
                tc.tile_pool(name="eres", bufs=1) as ep,
                tc.tile_pool(name="small", bufs=1) as sp,
                tc.tile_pool(name="pss", bufs=2, space="PSUM") as psp,
            ):
                zt = {}
                for name in PARAM_NAMES:
                    zt[name] = zp.tile([N, N], F32, tag=name, name=f"z_{name}")
                    nc.sync.dma_start(out=zt[name][:], in_=zs[name][:])

                def expm_transposed(z_tile, tag):
                    """Return expm(skew(Z))^T for the strict-upper skew of Z.

                    Maintains the (T, T^T) pair through Horner + squaring so
                    no PE transposes are needed: with negX = X^T = -X,
                      X @ T     = matmul(lhsT=negX, rhs=T)
                      T^T @ X^T = matmul(lhsT=T,    rhs=negX)
                    """
                    scal = 1.0 / (2.0 ** EXPM_SCAL)
                    us = wp.tile([N, N], F32, tag="us")
                    # us = (Z * scal) * mask  (strict upper part, prescaled)
                    nc.vector.scalar_tensor_tensor(
                        us[:], z_tile[:], scal, masku[:], op0=ALU.mult, op1=ALU.mult
                    )
                    pst = psp.tile([N, N], F32, tag="psA")
                    nc.tensor.transpose(pst[:], us[:], ident[:])
                    negx = wp.tile([N, N], F32, tag="negx")
                    # negX = us^T - us  ( = X^T = -X for X = us - us^T )
                    nc.vector.scalar_tensor_tensor(
                        negx[:], pst[:], 1.0, us[:], op0=ALU.mult, op1=ALU.subtract
                    )

                    t_cur, tt_cur = ident, ident
                    for j in range(EXPM_TERMS, 0, -1):
                        psa = psp.tile([N, N], F32, tag="psA")
                        psb = psp.tile([N, N], F32, tag="psB")
                        nc.tensor.matmul(
                            psa[:], negx[:], t_cur[:], start=True, stop=True
                        )
                        nc.tensor.matmul(
                            psb[:], t_cur[:], negx[:], start=True, stop=True
                        )
                        t_new = wp.tile([N, N], F32, tag="T")
                        tt_new = wp.tile([N, N], F32, tag="TT")
                        nc.vector.scalar_tensor_tensor(
                            t_new[:], psa[:], 1.0 / j, ident[:],
                            op0=ALU.mult, op1=ALU.add,
                        )
                        nc.vector.scalar_tensor_tensor(
                            tt_new[:], psb[:], 1.0 / j, ident[:],
                            op0=ALU.mult, op1=ALU.add,
                        )
                        t_cur, tt_cur = t_new, tt_new
                    for _ in range(EXPM_SCAL):
                        psa = psp.tile([N, N], F32, tag="psA")
                        psb = psp.tile([N, N], F32, tag="psB")
                        nc.tensor.matmul(
                            psa[:], tt_cur[:], t_cur[:], start=True, stop=True
                        )
                        nc.tensor.matmul(
                            psb[:], t_cur[:], tt_cur[:], start=True, stop=True
                        )
                        t_new = wp.tile([N, N], F32, tag="T")
                        tt_new = wp.tile([N, N], F32, tag="TT")
                        nc.vector.tensor_copy(t_new[:], psa[:])
                        nc.vector.tensor_copy(tt_new[:], psb[:])
                        t_cur, tt_cur = t_new, tt_new
                    res = ep.tile([N, N], F32, tag=tag)
                    nc.vector.tensor_copy(res[:], tt_cur[:])
                    return res

                uct = expm_transposed(zt["ZC_U"], "UCT")   # UC^T
                vct = expm_transposed(zt["ZC_V"], "VCT")   # VC^T
                ubt = expm_transposed(zt["ZB_U"], "UBT")   # UB^T
                vbt = expm_transposed(zt["ZB_V"], "VBT")   # VB^T
                uat = expm_transposed(zt["ZA_U"], "UAT")   # UA^T

                def absdiag_col(z_tile, tag):
                    tmp = wp.tile([N, N], F32, tag="us")
                    nc.vector.tensor_mul(tmp[:], z_tile[:], ident[:])
                    col = sp.tile([N, 1], F32, tag=tag)
                    nc.vector.tensor_reduce(
                        col[:], tmp[:], AXIS.X, ALU.add, apply_absolute_value=True
                    )
                    return col

                dc_col = absdiag_col(zt["ZC_S"], "dc")   # |diag(ZC_S)|
                db_col = absdiag_col(zt["ZB_S"], "db")   # |diag(ZB_S)|
                ga_col = absdiag_col(zt["ZA_G"], "ga")   # |diag(ZA_G)|

                # top-4: alpha = sqrt(sum_i (b_i c_i)^2), b/c sorted desc.
                bwork = sp.tile([N, 1], F32, tag="bwork")
                cwork = sp.tile([N, 1], F32, tag="cwork")
                nc.vector.tensor_copy(bwork[:], db_col[:])
                nc.vector.tensor_copy(cwork[:], dc_col[:])
                acc = sp.tile([N, 1], F32, tag="acc")
                nc.vector.memset(acc[:], 0.0)
                bmax = sp.tile([N, 1], F32, tag="bmax")
                cmax = sp.tile([N, 1], F32, tag="cmax")
                prod = sp.tile([N, 1], F32, tag="prod")
                gmask = sp.tile([N, 1], F32, tag="gmask")
                tdrop = sp.tile([N, 1], F32, tag="tdrop")
                for i in range(KTOP):
                    nc.gpsimd.partition_all_reduce(
                        bmax[:], bwork[:], N, bass_isa.ReduceOp.max
                    )
                    nc.gpsimd.partition_all_reduce(
                        cmax[:], cwork[:], N, bass_isa.ReduceOp.max
                    )
                    nc.vector.tensor_mul(prod[:], bmax[:], cmax[:])
                    nc.vector.tensor_mul(prod[:], prod[:], prod[:])
                    nc.vector.tensor_add(acc[:], acc[:], prod[:])
                    if i < KTOP - 1:
                        # zero out the extracted max (values all > 0)
                        nc.vector.tensor_single_scalar(
                            gmask[:], bwork[:], bmax[:], ALU.is_ge
                        )
                        nc.vector.tensor_mul(tdrop[:], bwork[:], gmask[:])
                        nc.vector.tensor_sub(bwork[:], bwork[:], tdrop[:])
                        nc.vector.tensor_single_scalar(
                            gmask[:], cwork[:], cmax[:], ALU.is_ge
                        )
                        nc.vector.tensor_mul(tdrop[:], cwork[:], gmask[:])
                        nc.vector.tensor_sub(cwork[:], cwork[:], tdrop[:])
                alpha = sp.tile([N, 1], F32, tag="alpha")
                nc.scalar.activation(alpha[:], acc[:], ACTF.Sqrt)

                # sa05 = -0.5*(alpha + gA)  (per-partition row scale of UA^T)
                sa05 = sp.tile([N, 1], F32, tag="sa05")
                nc.vector.tensor_scalar(
                    sa05[:], ga_col[:], alpha[:], -0.5, op0=ALU.add, op1=ALU.mult
                )
                sb01 = sp.tile([N, 1], F32, tag="sb01")
                nc.vector.tensor_scalar_mul(sb01[:], db_col[:], STEP)

                # C^T = VC @ (SC @ UC^T)
                p1 = wp.tile([N, N], F32, tag="us")
                nc.vector.tensor_scalar_mul(p1[:], uct[:], dc_col[:])
                psa = psp.tile([N, N], F32, tag="psA")
                nc.tensor.matmul(psa[:], vct[:], p1[:], start=True, stop=True)
                nc.vector.tensor_copy(CTm[:], psa[:])

                # (0.01 B)^T = VB @ (0.01 SB @ UB^T)
                p2 = wp.tile([N, N], F32, tag="us")
                nc.vector.tensor_scalar_mul(p2[:], ubt[:], sb01[:])
                psb = psp.tile([N, N], F32, tag="psB")
                nc.tensor.matmul(psb[:], vbt[:], p2[:], start=True, stop=True)
                nc.vector.tensor_copy(BpTm[:], psb[:])

                # M = UA @ (sa05 * UA^T) = 0.5*UA SA UA^T (symmetric)
                p3 = wp.tile([N, N], F32, tag="negx")
                nc.vector.tensor_scalar_mul(p3[:], uat[:], sa05[:])
                psm = psp.tile([N, N], F32, tag="psA")
                nc.tensor.matmul(psm[:], uat[:], p3[:], start=True, stop=True)
                # IYA = I - 0.005*YA,  YA = Uy - Uy^T
                uy = wp.tile([N, N], F32, tag="us")
                nc.vector.tensor_mul(uy[:], zt["ZA_Y"][:], masku[:])
                pst2 = psp.tile([N, N], F32, tag="psB")
                nc.tensor.transpose(pst2[:], uy[:], ident[:])
                q1 = wp.tile([N, N], F32, tag="T")
                # q1 = 0.005*Uy^T + I
                nc.vector.scalar_tensor_tensor(
                    q1[:], pst2[:], 0.5 * STEP, ident[:], op0=ALU.mult, op1=ALU.add
                )
                q2 = wp.tile([N, N], F32, tag="TT")
                # q2 = q1 - 0.005*Uy  (= I - 0.005*YA)
                nc.vector.tensor_scalar_mul(uy[:], uy[:], 0.5 * STEP)
                nc.vector.tensor_sub(q2[:], q1[:], uy[:])
                # ApT = 0.01*M + IYA  (= I + 0.01*A^T)
                nc.vector.scalar_tensor_tensor(
                    ApTm[:], psm[:], STEP, q2[:], op0=ALU.mult, op1=ALU.add
                )

            # ------- recurrence -------
            with (
                tc.tile_pool(name="xbuf", bufs=2) as xbufp,
                tc.tile_pool(name="stage", bufs=2) as stagep,
                tc.tile_pool(name="th", bufs=2 * groups) as thp,
                tc.tile_pool(name="psy", bufs=2, space="PSUM") as psyp,
                tc.tile_pool(name="psx", bufs=2, space="PSUM") as psxp,
                tc.tile_pool(name="pstr", bufs=2, space="PSUM") as pstrp,
            ):
                # xbuf column slot for local step s: pairs (i, i+half) are
                # adjacent so the PE transpose reads one contiguous block
                # (walrus: matmul weight APs must have a single free dim).
                def slot(s):
                    return 2 * (s % half) + (s // half)

                xb_prev = None
                for c in range(nchunks):
                    xb = xbufp.tile([N, tc_chunk * BSH], rdt, tag="xb")
                    st = stagep.tile([128, half * N], F32, tag="st")
                    if c == 0:
                        nc.vector.tensor_copy(xb[:, 0:BSH], x0_c[:])
                    for s in range(tc_chunk):
                        t = c * tc_chunk + s
                        if t > 0:
                            if s > 0:
                                pxb, ps_ = xb, slot(s - 1)
                            else:
                                pxb, ps_ = xb_prev, slot(tc_chunk - 1)
                            for g in range(groups):
                                xprev = pxb[:, ds(ps_ * BSH + g * gcols, gcols)]
                                psy = psyp.tile([N, gcols], F32, tag="psy")
                                nc.tensor.matmul(
                                    psy[:], CTm[:], xprev, start=True, stop=True
                                )
                                psx = psxp.tile([N, gcols], F32, tag="psx")
                                nc.tensor.matmul(
                                    psx[:], ApTm[:], xprev, start=True, stop=False
                                )
                                th = thp.tile([N, gcols], rdt, tag="th")
                                nc.scalar.activation(
                                    th[:], psy[:], ACTF.Tanh, bias=by_c[:], scale=1.0
                                )
                                nc.tensor.matmul(
                                    psx[:], BpTm[:], th[:], start=False, stop=True
                                )
                                nc.vector.tensor_scalar_add(
                                    xb[:, ds(slot(s) * BSH + g * gcols, gcols)],
                                    psx[:],
                                    bxp_c[:],
                                )
                        if s >= half:
                            i = s - half
                            # transpose steps (i, i+half): adjacent slots
                            # (2i, 2i+1) -> one contiguous 128-col block
                            pstr = pstrp.tile([128, N], rdt, tag="pstr")
                            nc.tensor.transpose(
                                pstr[:], xb[:, ds(2 * i * BSH, 2 * BSH)], ident_r[:]
                            )
                            nc.vector.tensor_copy(st[:, ds(i * N, N)], pstr[:])
                    for h in range(2):
                        t0 = c * tc_chunk + h * half
                        dram_ap = out[:, t0:t0 + half, :].rearrange(
                            "b i n -> b (i n)"
                        )
                        nc.sync.dma_start(
                            out=dram_ap, in_=st[h * 64:(h + 1) * 64, :]
                        )
                    xb_prev = xb

    nc.compile()
    return nc


_CACHED = {}


def _get_program(tmax=TMAX, tc_chunk=32, groups=2, rdt=F32R):
    key = (tmax, tc_chunk, groups, str(rdt))
    if key not in _CACHED:
        _CACHED[key] = build_program(tmax, tc_chunk, groups, rdt)
    return _CACHED[key]


def make_in_maps(inputs, tmax=TMAX):
    X0 = np.ascontiguousarray(np.asarray(inputs["X0"], dtype=np.float32))
    base = {
        name: np.ascontiguousarray(np.asarray(inputs[name], dtype=np.float32))
        for name in PARAM_NAMES
    }
    base["bx"] = np.ascontiguousarray(
        np.asarray(inputs["bx"], dtype=np.float32).reshape(N, 1)
    )
    base["by"] = np.ascontiguousarray(
        np.asarray(inputs["by"], dtype=np.float32).reshape(N, 1)
    )
    in_maps = []
    for c in range(NCORES):
        m = dict(base)
        m["x0"] = np.ascontiguousarray(X0[c * BSH:(c + 1) * BSH].T)
        in_maps.append(m)
    return in_maps


def run_spmd(inputs, tmax=TMAX, tc_chunk=32, groups=2, trace=False, tmpdir=None,
             rdt=F32R):
    nc = _get_program(tmax, tc_chunk, groups, rdt)
    in_maps = make_in_maps(inputs, tmax)
    res = run_bass_kernel_spmd(
        nc, in_maps, list(range(NCORES)), trace=trace, tmpdir=tmpdir
    )
    outs = [res.results[c]["out"] for c in range(NCORES)]
    full = np.concatenate(outs, axis=0)
    return full, res


def kernel(**inputs):
    full, _ = run_spmd(inputs)
    return full


# revision 9
# speedup vs baseline: 2.0118x; 1.5191x over previous
"""LurieNet-k Trainium2 kernel.

Computes, from the raw parametrization tensors, the matrices
  C = UC @ SC @ VC^T,  B = UB @ SB @ VB^T,
  A = 0.5*UA @ SA @ UA^T + 0.5*YA  (SA = -(alpha_upp*I + GA))
entirely on device (matrix exponentials of skew matrices via
scaling-and-squaring Taylor), then runs the 511-step recurrence
  y  = C x + by
  x' = x + (0.01*A x + 0.01*B tanh(y) + 0.01*bx)
on a (128, 64) state shard per NeuronCore (batch data-parallel over the
8 cores), writing the full (b, t, n) trajectory.

Precision: the system amplifies per-step state perturbations, so the
identity part of the state update is carried in fp32 OUTSIDE the
matmuls (split-carry): the matmuls only apply gains << 1 (C ~ 0.3,
0.01A ~ 0.003, 0.01B ~ 0.003), so their operands can be bf16 without
error blowup (measured ~2e-3 final rel err vs fp32's 6e-6; naive bf16
with A' = I + 0.01A as a matmul diverges to 0.36). bf16 weights get
FWL weight loads, which matters because the kernel is otherwise
LDWEIGHTS-throughput bound.
"""

import sys

for _p in ("/opt/trn_rl_repo",):
    if _p not in sys.path:
        sys.path.insert(0, _p)

import numpy as np

import concourse.bass as bass
import concourse.mybir as mybir
import concourse.tile as tile
from concourse import bacc
from concourse import bass_isa
from concourse.bass import ds
from concourse.bass_utils import run_bass_kernel_spmd
from concourse.masks import make_identity, make_upper_triangular

F32 = mybir.dt.float32
BF16 = mybir.dt.bfloat16
ALU = mybir.AluOpType
ACTF = mybir.ActivationFunctionType
AXIS = mybir.AxisListType

N = 128          # state dim
TMAX = 512       # time steps (including t=0)
BS = 512         # global batch
NCORES = 8
BSH = BS // NCORES   # 64 batch columns per core
STEP = 0.01
KTOP = 4

EXPM_SCAL = 6    # expm scaling: X = S / 2**EXPM_SCAL, then 6 squarings
EXPM_TERMS = 7   # Taylor terms in the Horner evaluation

PARAM_NAMES = [
    "ZA_Y", "ZA_U", "ZA_G", "ZB_U", "ZB_V", "ZB_S", "ZC_U", "ZC_V", "ZC_S",
]


def build_program(tmax=TMAX, tc_chunk=32, mdt=BF16):
    """Build the single-NeuronCore Bass program (run SPMD on all 8 cores).

    mdt: dtype of the recurrence matmul operands (weights, rounded state
    copy, tanh output). The fp32 state carry is exact regardless.
    """
    assert tmax % tc_chunk == 0 and tc_chunk % 2 == 0
    half = tc_chunk // 2
    nchunks = tmax // tc_chunk

    nc = bacc.Bacc(
        "TRN2",
        target_bir_lowering=False,
        debug=False,
        enable_asserts=False,
        num_devices=NCORES,
    )

    x0 = nc.dram_tensor("x0", [N, BSH], F32, kind="ExternalInput")
    zs = {
        name: nc.dram_tensor(name, [N, N], F32, kind="ExternalInput")
        for name in PARAM_NAMES
    }
    bx_d = nc.dram_tensor("bx", [N, 1], F32, kind="ExternalInput")
    by_d = nc.dram_tensor("by", [N, 1], F32, kind="ExternalInput")
    out = nc.dram_tensor("out", [BSH, tmax, N], F32, kind="ExternalOutput")

    with tile.TileContext(nc) as tc:
        with tc.tile_pool(name="const", bufs=1) as constp:
            ident = constp.tile([N, N], F32, tag="ident")
            make_identity(nc, ident[:])
            masku = constp.tile([N, N], F32, tag="masku")
            make_upper_triangular(nc, masku[:], val=1.0, diag=False)

            by_c = constp.tile([N, 1], F32, tag="by")
            nc.sync.dma_start(out=by_c[:], in_=by_d[:])
            bx_c = constp.tile([N, 1], F32, tag="bxraw")
            nc.sync.dma_start(out=bx_c[:], in_=bx_d[:])
            bxp_c = constp.tile([N, 1], F32, tag="bxp")
            nc.vector.tensor_scalar_mul(bxp_c[:], bx_c[:], STEP)
            x0_c = constp.tile([N, BSH], F32, tag="x0c")
            nc.sync.dma_start(out=x0_c[:], in_=x0[:])

            # ------- setup phase: expm's + weight assembly -------
            CTm = constp.tile([N, N], mdt, tag="CTm")     # C^T
            A01Tm = constp.tile([N, N], mdt, tag="A01Tm")  # (0.01 A)^T
            BpTm = constp.tile([N, N], mdt, tag="BpTm")   # (0.01 B)^T

            with (
                tc.tile_pool(name="zbuf", bufs=1) as zp,
                tc.tile_pool(name="work", bufs=2) as wp,
                tc.tile_pool(name="eres", bufs=1) as ep,
                tc.tile_pool(name="small", bufs=1) as sp,
                tc.tile_pool(name="pss", bufs=2, space="PSUM") as psp,
            ):
                zt = {}
                for name in PARAM_NAMES:
                    zt[name] = zp.tile([N, N], F32, tag=name, name=f"z_{name}")
                    nc.sync.dma_start(out=zt[name][:], in_=zs[name][:])

                def expm_transposed(z_tile, tag):
                    """Return expm(skew(Z))^T for the strict-upper skew of Z.

                    Maintains the (T, T^T) pair through Horner + squaring so
                    no PE transposes are needed: with negX = X^T = -X,
                      X @ T     = matmul(lhsT=negX, rhs=T)
                      T^T @ X^T = matmul(lhsT=T,    rhs=negX)
                    """
                    scal = 1.0 / (2.0 ** EXPM_SCAL)
                    us = wp.tile([N, N], F32, tag="us")
                    # us = (Z * scal) * mask  (strict upper part, prescaled)
                    nc.vector.scalar_tensor_tensor(
                        us[:], z_tile[:], scal, masku[:], op0=ALU.mult, op1=ALU.mult
                    )
                    pst = psp.tile([N, N], F32, tag="psA")
                    nc.tensor.transpose(pst[:], us[:], ident[:])
                    negx = wp.tile([N, N], F32, tag="negx")
                    # negX = us^T - us  ( = X^T = -X for X = us - us^T )
                    nc.vector.scalar_tensor_tensor(
                        negx[:], pst[:], 1.0, us[:], op0=ALU.mult, op1=ALU.subtract
                    )

                    t_cur, tt_cur = ident, ident
                    for j in range(EXPM_TERMS, 0, -1):
                        psa = psp.tile([N, N], F32, tag="psA")
                        psb = psp.tile([N, N], F32, tag="psB")
                        nc.tensor.matmul(
                            psa[:], negx[:], t_cur[:], start=True, stop=True
                        )
                        nc.tensor.matmul(
                            psb[:], t_cur[:], negx[:], start=True, stop=True
                        )
                        t_new = wp.tile([N, N], F32, tag="T")
                        tt_new = wp.tile([N, N], F32, tag="TT")
                        nc.vector.scalar_tensor_tensor(
                            t_new[:], psa[:], 1.0 / j, ident[:],
                            op0=ALU.mult, op1=ALU.add,
                        )
                        nc.vector.scalar_tensor_tensor(
                            tt_new[:], psb[:], 1.0 / j, ident[:],
                            op0=ALU.mult, op1=ALU.add,
                        )
                        t_cur, tt_cur = t_new, tt_new
                    for _ in range(EXPM_SCAL):
                        psa = psp.tile([N, N], F32, tag="psA")
                        psb = psp.tile([N, N], F32, tag="psB")
                        nc.tensor.matmul(
                            psa[:], tt_cur[:], t_cur[:], start=True, stop=True
                        )
                        nc.tensor.matmul(
                            psb[:], t_cur[:], tt_cur[:], start=True, stop=True
                        )
                        t_new = wp.tile([N, N], F32, tag="T")
                        tt_new = wp.tile([N, N], F32, tag="TT")
                        nc.vector.tensor_copy(t_new[:], psa[:])
                        nc.vector.tensor_copy(tt_new[:], psb[:])
                        t_cur, tt_cur = t_new, tt_new
                    res = ep.tile([N, N], F32, tag=tag)
                    nc.vector.tensor_copy(res[:], tt_cur[:])
                    return res

                uct = expm_transposed(zt["ZC_U"], "UCT")   # UC^T
                vct = expm_transposed(zt["ZC_V"], "VCT")   # VC^T
                ubt = expm_transposed(zt["ZB_U"], "UBT")   # UB^T
                vbt = expm_transposed(zt["ZB_V"], "VBT")   # VB^T
                uat = expm_transposed(zt["ZA_U"], "UAT")   # UA^T

                def absdiag_col(z_tile, tag):
                    tmp = wp.tile([N, N], F32, tag="us")
                    nc.vector.tensor_mul(tmp[:], z_tile[:], ident[:])
                    col = sp.tile([N, 1], F32, tag=tag, name=f"col_{tag}")
                    nc.vector.tensor_reduce(
                        col[:], tmp[:], AXIS.X, ALU.add, apply_absolute_value=True
                    )
                    return col

                dc_col = absdiag_col(zt["ZC_S"], "dc")   # |diag(ZC_S)|
                db_col = absdiag_col(zt["ZB_S"], "db")   # |diag(ZB_S)|
                ga_col = absdiag_col(zt["ZA_G"], "ga")   # |diag(ZA_G)|

                # top-4: alpha = sqrt(sum_i (b_i c_i)^2), b/c sorted desc.
                bwork = sp.tile([N, 1], F32, tag="bwork")
                cwork = sp.tile([N, 1], F32, tag="cwork")
                nc.vector.tensor_copy(bwork[:], db_col[:])
                nc.vector.tensor_copy(cwork[:], dc_col[:])
                acc = sp.tile([N, 1], F32, tag="acc")
                nc.vector.memset(acc[:], 0.0)
                bmax = sp.tile([N, 1], F32, tag="bmax")
                cmax = sp.tile([N, 1], F32, tag="cmax")
                prod = sp.tile([N, 1], F32, tag="prod")
                gmask = sp.tile([N, 1], F32, tag="gmask")
                tdrop = sp.tile([N, 1], F32, tag="tdrop")
                for i in range(KTOP):
                    nc.gpsimd.partition_all_reduce(
                        bmax[:], bwork[:], N, bass_isa.ReduceOp.max
                    )
                    nc.gpsimd.partition_all_reduce(
                        cmax[:], cwork[:], N, bass_isa.ReduceOp.max
                    )
                    nc.vector.tensor_mul(prod[:], bmax[:], cmax[:])
                    nc.vector.tensor_mul(prod[:], prod[:], prod[:])
                    nc.vector.tensor_add(acc[:], acc[:], prod[:])
                    if i < KTOP - 1:
                        # zero out the extracted max (values all > 0)
                        nc.vector.tensor_single_scalar(
                            gmask[:], bwork[:], bmax[:], ALU.is_ge
                        )
                        nc.vector.tensor_mul(tdrop[:], bwork[:], gmask[:])
                        nc.vector.tensor_sub(bwork[:], bwork[:], tdrop[:])
                        nc.vector.tensor_single_scalar(
                            gmask[:], cwork[:], cmax[:], ALU.is_ge
                        )
                        nc.vector.tensor_mul(tdrop[:], cwork[:], gmask[:])
                        nc.vector.tensor_sub(cwork[:], cwork[:], tdrop[:])
                alpha = sp.tile([N, 1], F32, tag="alpha")
                nc.scalar.activation(alpha[:], acc[:], ACTF.Sqrt)

                # sa05 = -0.5*(alpha + gA)  (per-partition row scale of UA^T)
                sa05 = sp.tile([N, 1], F32, tag="sa05")
                nc.vector.tensor_scalar(
                    sa05[:], ga_col[:], alpha[:], -0.5, op0=ALU.add, op1=ALU.mult
                )
                sb01 = sp.tile([N, 1], F32, tag="sb01")
                nc.vector.tensor_scalar_mul(sb01[:], db_col[:], STEP)

                # C^T = VC @ (SC @ UC^T)
                p1 = wp.tile([N, N], F32, tag="us")
                nc.vector.tensor_scalar_mul(p1[:], uct[:], dc_col[:])
                psa = psp.tile([N, N], F32, tag="psA")
                nc.tensor.matmul(psa[:], vct[:], p1[:], start=True, stop=True)
                nc.vector.tensor_copy(CTm[:], psa[:])

                # (0.01 B)^T = VB @ (0.01 SB @ UB^T)
                p2 = wp.tile([N, N], F32, tag="us")
                nc.vector.tensor_scalar_mul(p2[:], ubt[:], sb01[:])
                psb = psp.tile([N, N], F32, tag="psB")
                nc.tensor.matmul(psb[:], vbt[:], p2[:], start=True, stop=True)
                nc.vector.tensor_copy(BpTm[:], psb[:])

                # M = UA @ (sa05 * UA^T) = 0.5*UA SA UA^T (symmetric)
                p3 = wp.tile([N, N], F32, tag="negx")
                nc.vector.tensor_scalar_mul(p3[:], uat[:], sa05[:])
                psm = psp.tile([N, N], F32, tag="psA")
                nc.tensor.matmul(psm[:], uat[:], p3[:], start=True, stop=True)
                # (0.01 A)^T = 0.01*M - 0.005*YA,  YA = Uy - Uy^T
                uy = wp.tile([N, N], F32, tag="us")
                nc.vector.tensor_mul(uy[:], zt["ZA_Y"][:], masku[:])
                pst2 = psp.tile([N, N], F32, tag="psB")
                nc.tensor.transpose(pst2[:], uy[:], ident[:])
                nc.vector.tensor_scalar_mul(uy[:], uy[:], 0.5 * STEP)
                q2 = wp.tile([N, N], F32, tag="T")
                # q2 = 0.005*Uy^T - 0.005*Uy  (= -0.005*YA)
                nc.vector.scalar_tensor_tensor(
                    q2[:], pst2[:], 0.5 * STEP, uy[:], op0=ALU.mult, op1=ALU.subtract
                )
                nc.vector.scalar_tensor_tensor(
                    A01Tm[:], psm[:], STEP, q2[:], op0=ALU.mult, op1=ALU.add
                )

            # ------- recurrence (split-carry) -------
            with (
                tc.tile_pool(name="xbuf", bufs=2) as xbufp,
                tc.tile_pool(name="stage", bufs=2) as stagep,
                tc.tile_pool(name="xr", bufs=3) as xrp,
                tc.tile_pool(name="th", bufs=3) as thp,
                tc.tile_pool(name="psy", bufs=3, space="PSUM") as psyp,
                tc.tile_pool(name="psx", bufs=3, space="PSUM") as psxp,
                tc.tile_pool(name="pstr", bufs=2, space="PSUM") as pstrp,
            ):
                # xbuf column slot for local step s: pairs (i, i+half) are
                # adjacent so the PE transpose reads one contiguous block
                # (walrus: matmul weight APs must have a single free dim).
                def slot(s):
                    return 2 * (s % half) + (s // half)

                # rounded copy of the state for matmul consumption
                xr_prev = xrp.tile([N, BSH], mdt, tag="xr", name="xr_init")
                nc.vector.tensor_copy(xr_prev[:], x0_c[:])

                xb_prev = None
                for c in range(nchunks):
                    xb = xbufp.tile([N, tc_chunk * BSH], F32, tag="xb")
                    st = stagep.tile([128, half * N], F32, tag="st")
                    if c == 0:
                        nc.vector.tensor_copy(xb[:, 0:BSH], x0_c[:])
                    for s in range(tc_chunk):
                        t = c * tc_chunk + s
                        if t > 0:
                            if s > 0:
                                pxb, ps_ = xb, slot(s - 1)
                            else:
                                pxb, ps_ = xb_prev, slot(tc_chunk - 1)
                            xprev_f32 = pxb[:, ds(ps_ * BSH, BSH)]
                            psy = psyp.tile([N, BSH], F32, tag="psy")
                            nc.tensor.matmul(
                                psy[:], CTm[:], xr_prev[:], start=True, stop=True
                            )
                            psx = psxp.tile([N, BSH], F32, tag="psx")
                            nc.tensor.matmul(
                                psx[:], A01Tm[:], xr_prev[:], start=True, stop=False
                            )
                            th = thp.tile([N, BSH], mdt, tag="th")
                            nc.scalar.activation(
                                th[:], psy[:], ACTF.Tanh, bias=by_c[:], scale=1.0
                            )
                            nc.tensor.matmul(
                                psx[:], BpTm[:], th[:], start=False, stop=True
                            )
                            # chain-critical: rounded next state for the matmuls
                            xr_new = xrp.tile([N, BSH], mdt, tag="xr")
                            nc.vector.scalar_tensor_tensor(
                                xr_new[:], psx[:], bxp_c[:], xprev_f32,
                                op0=ALU.add, op1=ALU.add,
                            )
                            # exact fp32 state carry (also the output value)
                            nc.vector.scalar_tensor_tensor(
                                xb[:, ds(slot(s) * BSH, BSH)], psx[:], bxp_c[:],
                                xprev_f32, op0=ALU.add, op1=ALU.add,
                            )
                            xr_prev = xr_new
                        if s >= half:
                            i = s - half
                            # transpose steps (i, i+half): adjacent slots
                            # (2i, 2i+1) -> one contiguous 128-col block
                            pstr = pstrp.tile([128, N], F32, tag="pstr")
                            nc.tensor.transpose(
                                pstr[:], xb[:, ds(2 * i * BSH, 2 * BSH)], ident[:]
                            )
                            nc.scalar.copy(st[:, ds(i * N, N)], pstr[:])
                    for h in range(2):
                        t0 = c * tc_chunk + h * half
                        dram_ap = out[:, t0:t0 + half, :].rearrange(
                            "b i n -> b (i n)"
                        )
                        nc.sync.dma_start(
                            out=dram_ap, in_=st[h * 64:(h + 1) * 64, :]
                        )
                    xb_prev = xb

    nc.compile()
    return nc


_CACHED = {}


def _get_program(tmax=TMAX, tc_chunk=32, mdt=BF16):
    key = (tmax, tc_chunk, str(mdt))
    if key not in _CACHED:
        _CACHED[key] = build_program(tmax, tc_chunk, mdt)
    return _CACHED[key]


def make_in_maps(inputs, tmax=TMAX):
    X0 = np.ascontiguousarray(np.asarray(inputs["X0"], dtype=np.float32))
    base = {
        name: np.ascontiguousarray(np.asarray(inputs[name], dtype=np.float32))
        for name in PARAM_NAMES
    }
    base["bx"] = np.ascontiguousarray(
        np.asarray(inputs["bx"], dtype=np.float32).reshape(N, 1)
    )
    base["by"] = np.ascontiguousarray(
        np.asarray(inputs["by"], dtype=np.float32).reshape(N, 1)
    )
    in_maps = []
    for c in range(NCORES):
        m = dict(base)
        m["x0"] = np.ascontiguousarray(X0[c * BSH:(c + 1) * BSH].T)
        in_maps.append(m)
    return in_maps


def run_spmd(inputs, tmax=TMAX, tc_chunk=32, trace=False, tmpdir=None, mdt=BF16):
    nc = _get_program(tmax, tc_chunk, mdt)
    in_maps = make_in_maps(inputs, tmax)
    res = run_bass_kernel_spmd(
        nc, in_maps, list(range(NCORES)), trace=trace, tmpdir=tmpdir
    )
    outs = [res.results[c]["out"] for c in range(NCORES)]
    full = np.concatenate(outs, axis=0)
    return full, res


def kernel(**inputs):
    full, _ = run_spmd(inputs)
    return full


# revision 12
# speedup vs baseline: 2.9437x; 1.4632x over previous
"""LurieNet-k Trainium2 kernel.

Computes, from the raw parametrization tensors, the matrices
  C = UC @ SC @ VC^T,  B = UB @ SB @ VB^T,
  A = 0.5*UA @ SA @ UA^T + 0.5*YA  (SA = -(alpha_upp*I + GA))
entirely on device (matrix exponentials of skew matrices via
scaling-and-squaring Taylor), then runs the 511-step recurrence
  y  = C x + by
  x' = x + (0.01*A x + 0.01*B tanh(y) + 0.01*bx)
on a (128, 64) state shard per NeuronCore (batch data-parallel over the
8 cores), writing the full (b, t, n) trajectory.

Precision: the system amplifies per-step state perturbations, so the
identity part of the state update is carried in fp32 OUTSIDE the
matmuls (split-carry): the matmuls only apply gains << 1 (C ~ 0.3,
0.01A ~ 0.003, 0.01B ~ 0.003), so their operands can be bf16 without
error blowup (measured ~2e-3 final rel err vs fp32's 6e-6; naive bf16
with A' = I + 0.01A as a matmul diverges to 0.36). bf16 weights get
FWL weight loads, which matters because the kernel is otherwise
LDWEIGHTS-throughput bound.
"""

import sys

for _p in ("/opt/trn_rl_repo",):
    if _p not in sys.path:
        sys.path.insert(0, _p)

import numpy as np

import concourse.bass as bass
import concourse.mybir as mybir
import concourse.tile as tile
from concourse import bacc
from concourse import bass_isa
from concourse.bass import ds
from concourse.bass_utils import run_bass_kernel_spmd
from concourse.masks import make_identity, make_upper_triangular

F32 = mybir.dt.float32
BF16 = mybir.dt.bfloat16
ALU = mybir.AluOpType
ACTF = mybir.ActivationFunctionType
AXIS = mybir.AxisListType

N = 128          # state dim
TMAX = 512       # time steps (including t=0)
BS = 512         # global batch
NCORES = 8
BSH = BS // NCORES   # 64 batch columns per core
STEP = 0.01
KTOP = 4

EXPM_SCAL = 6    # expm scaling: X = S / 2**EXPM_SCAL, then 6 squarings
EXPM_TERMS = 7   # Taylor terms in the Horner evaluation

PARAM_NAMES = [
    "ZA_Y", "ZA_U", "ZA_G", "ZB_U", "ZB_V", "ZB_S", "ZC_U", "ZC_V", "ZC_S",
]


def build_program(tmax=TMAX, tc_chunk=32, mdt=BF16):
    """Build the single-NeuronCore Bass program (run SPMD on all 8 cores).

    mdt: dtype of the recurrence matmul operands (weights, rounded state
    copy, tanh output). The fp32 state carry is exact regardless.
    """
    assert tmax % tc_chunk == 0 and tc_chunk % 2 == 0
    half = tc_chunk // 2
    nchunks = tmax // tc_chunk

    nc = bacc.Bacc(
        "TRN2",
        target_bir_lowering=False,
        debug=False,
        enable_asserts=False,
        num_devices=NCORES,
    )

    x0 = nc.dram_tensor("x0", [N, BSH], F32, kind="ExternalInput")
    zs = {
        name: nc.dram_tensor(name, [N, N], F32, kind="ExternalInput")
        for name in PARAM_NAMES
    }
    bx_d = nc.dram_tensor("bx", [N, 1], F32, kind="ExternalInput")
    by_d = nc.dram_tensor("by", [N, 1], F32, kind="ExternalInput")
    out = nc.dram_tensor("out", [BSH, tmax, N], F32, kind="ExternalOutput")

    with tile.TileContext(nc) as tc:
        with tc.tile_pool(name="const", bufs=1) as constp:
            ident = constp.tile([N, N], F32, tag="ident")
            make_identity(nc, ident[:])
            masku = constp.tile([N, N], F32, tag="masku")
            make_upper_triangular(nc, masku[:], val=1.0, diag=False)

            by_c = constp.tile([N, 1], F32, tag="by")
            nc.sync.dma_start(out=by_c[:], in_=by_d[:])
            bx_c = constp.tile([N, 1], F32, tag="bxraw")
            nc.sync.dma_start(out=bx_c[:], in_=bx_d[:])
            bxp_c = constp.tile([N, 1], F32, tag="bxp")
            nc.vector.tensor_scalar_mul(bxp_c[:], bx_c[:], STEP)
            x0_c = constp.tile([N, BSH], F32, tag="x0c")
            nc.sync.dma_start(out=x0_c[:], in_=x0[:])

            # ------- setup phase: expm's + weight assembly -------
            # Recurrence uses the P-form to split the serial chain in two:
            #   y_t  = P x_{t-1} + Q th_{t-1} + r,  th_t = tanh(y_t)
            #   x_t  = x_{t-1} + (0.01A) x_{t-1} + (0.01B) th_{t-1} + 0.01 bx
            # with P = C + 0.01 C A, Q = 0.01 C B, r = 0.01 C bx + by.
            PTm = constp.tile([N, N], mdt, tag="PTm")      # P^T
            QTm = constp.tile([N, N], mdt, tag="QTm")      # Q^T
            A01Tm = constp.tile([N, N], mdt, tag="A01Tm")  # (0.01 A)^T
            BpTm = constp.tile([N, N], mdt, tag="BpTm")    # (0.01 B)^T
            CTf32 = constp.tile([N, N], F32, tag="CTf32")  # C^T fp32
            r_c = constp.tile([N, 1], F32, tag="rc")       # 0.01 C bx + by

            with (
                tc.tile_pool(name="zbuf", bufs=1) as zp,
                tc.tile_pool(name="work", bufs=2) as wp,
                tc.tile_pool(name="eres", bufs=1) as ep,
                tc.tile_pool(name="small", bufs=1) as sp,
                tc.tile_pool(name="pss", bufs=2, space="PSUM") as psp,
            ):
                zt = {}
                for name in PARAM_NAMES:
                    zt[name] = zp.tile([N, N], F32, tag=name, name=f"z_{name}")
                    nc.sync.dma_start(out=zt[name][:], in_=zs[name][:])

                def expm_transposed(z_tile, tag):
                    """Return expm(skew(Z))^T for the strict-upper skew of Z.

                    Maintains the (T, T^T) pair through Horner + squaring so
                    no PE transposes are needed: with negX = X^T = -X,
                      X @ T     = matmul(lhsT=negX, rhs=T)
                      T^T @ X^T = matmul(lhsT=T,    rhs=negX)
                    """
                    scal = 1.0 / (2.0 ** EXPM_SCAL)
                    us = wp.tile([N, N], F32, tag="us")
                    # us = (Z * scal) * mask  (strict upper part, prescaled)
                    nc.vector.scalar_tensor_tensor(
                        us[:], z_tile[:], scal, masku[:], op0=ALU.mult, op1=ALU.mult
                    )
                    pst = psp.tile([N, N], F32, tag="psA")
                    nc.tensor.transpose(pst[:], us[:], ident[:])
                    negx = wp.tile([N, N], F32, tag="negx")
                    # negX = us^T - us  ( = X^T = -X for X = us - us^T )
                    nc.vector.scalar_tensor_tensor(
                        negx[:], pst[:], 1.0, us[:], op0=ALU.mult, op1=ALU.subtract
                    )

                    t_cur, tt_cur = ident, ident
                    for j in range(EXPM_TERMS, 0, -1):
                        psa = psp.tile([N, N], F32, tag="psA")
                        psb = psp.tile([N, N], F32, tag="psB")
                        nc.tensor.matmul(
                            psa[:], negx[:], t_cur[:], start=True, stop=True
                        )
                        nc.tensor.matmul(
                            psb[:], t_cur[:], negx[:], start=True, stop=True
                        )
                        t_new = wp.tile([N, N], F32, tag="T")
                        tt_new = wp.tile([N, N], F32, tag="TT")
                        nc.vector.scalar_tensor_tensor(
                            t_new[:], psa[:], 1.0 / j, ident[:],
                            op0=ALU.mult, op1=ALU.add,
                        )
                        nc.vector.scalar_tensor_tensor(
                            tt_new[:], psb[:], 1.0 / j, ident[:],
                            op0=ALU.mult, op1=ALU.add,
                        )
                        t_cur, tt_cur = t_new, tt_new
                    for _ in range(EXPM_SCAL):
                        psa = psp.tile([N, N], F32, tag="psA")
                        psb = psp.tile([N, N], F32, tag="psB")
                        nc.tensor.matmul(
                            psa[:], tt_cur[:], t_cur[:], start=True, stop=True
                        )
                        nc.tensor.matmul(
                            psb[:], t_cur[:], tt_cur[:], start=True, stop=True
                        )
                        t_new = wp.tile([N, N], F32, tag="T")
                        tt_new = wp.tile([N, N], F32, tag="TT")
                        nc.vector.tensor_copy(t_new[:], psa[:])
                        nc.vector.tensor_copy(tt_new[:], psb[:])
                        t_cur, tt_cur = t_new, tt_new
                    res = ep.tile([N, N], F32, tag=tag)
                    nc.vector.tensor_copy(res[:], tt_cur[:])
                    return res

                uct = expm_transposed(zt["ZC_U"], "UCT")   # UC^T
                vct = expm_transposed(zt["ZC_V"], "VCT")   # VC^T
                ubt = expm_transposed(zt["ZB_U"], "UBT")   # UB^T
                vbt = expm_transposed(zt["ZB_V"], "VBT")   # VB^T
                uat = expm_transposed(zt["ZA_U"], "UAT")   # UA^T

                def absdiag_col(z_tile, tag):
                    tmp = wp.tile([N, N], F32, tag="us")
                    nc.vector.tensor_mul(tmp[:], z_tile[:], ident[:])
                    col = sp.tile([N, 1], F32, tag=tag, name=f"col_{tag}")
                    nc.vector.tensor_reduce(
                        col[:], tmp[:], AXIS.X, ALU.add, apply_absolute_value=True
                    )
                    return col

                dc_col = absdiag_col(zt["ZC_S"], "dc")   # |diag(ZC_S)|
                db_col = absdiag_col(zt["ZB_S"], "db")   # |diag(ZB_S)|
                ga_col = absdiag_col(zt["ZA_G"], "ga")   # |diag(ZA_G)|

                # top-4: alpha = sqrt(sum_i (b_i c_i)^2), b/c sorted desc.
                bwork = sp.tile([N, 1], F32, tag="bwork")
                cwork = sp.tile([N, 1], F32, tag="cwork")
                nc.vector.tensor_copy(bwork[:], db_col[:])
                nc.vector.tensor_copy(cwork[:], dc_col[:])
                acc = sp.tile([N, 1], F32, tag="acc")
                nc.vector.memset(acc[:], 0.0)
                bmax = sp.tile([N, 1], F32, tag="bmax")
                cmax = sp.tile([N, 1], F32, tag="cmax")
                prod = sp.tile([N, 1], F32, tag="prod")
                gmask = sp.tile([N, 1], F32, tag="gmask")
                tdrop = sp.tile([N, 1], F32, tag="tdrop")
                for i in range(KTOP):
                    nc.gpsimd.partition_all_reduce(
                        bmax[:], bwork[:], N, bass_isa.ReduceOp.max
                    )
                    nc.gpsimd.partition_all_reduce(
                        cmax[:], cwork[:], N, bass_isa.ReduceOp.max
                    )
                    nc.vector.tensor_mul(prod[:], bmax[:], cmax[:])
                    nc.vector.tensor_mul(prod[:], prod[:], prod[:])
                    nc.vector.tensor_add(acc[:], acc[:], prod[:])
                    if i < KTOP - 1:
                        # zero out the extracted max (values all > 0)
                        nc.vector.tensor_single_scalar(
                            gmask[:], bwork[:], bmax[:], ALU.is_ge
                        )
                        nc.vector.tensor_mul(tdrop[:], bwork[:], gmask[:])
                        nc.vector.tensor_sub(bwork[:], bwork[:], tdrop[:])
                        nc.vector.tensor_single_scalar(
                            gmask[:], cwork[:], cmax[:], ALU.is_ge
                        )
                        nc.vector.tensor_mul(tdrop[:], cwork[:], gmask[:])
                        nc.vector.tensor_sub(cwork[:], cwork[:], tdrop[:])
                alpha = sp.tile([N, 1], F32, tag="alpha")
                nc.scalar.activation(alpha[:], acc[:], ACTF.Sqrt)

                # sa05 = -0.5*(alpha + gA)  (per-partition row scale of UA^T)
                sa05 = sp.tile([N, 1], F32, tag="sa05")
                nc.vector.tensor_scalar(
                    sa05[:], ga_col[:], alpha[:], -0.5, op0=ALU.add, op1=ALU.mult
                )
                sb01 = sp.tile([N, 1], F32, tag="sb01")
                nc.vector.tensor_scalar_mul(sb01[:], db_col[:], STEP)

                # C^T = VC @ (SC @ UC^T)
                p1 = wp.tile([N, N], F32, tag="us")
                nc.vector.tensor_scalar_mul(p1[:], uct[:], dc_col[:])
                psa = psp.tile([N, N], F32, tag="psA")
                nc.tensor.matmul(psa[:], vct[:], p1[:], start=True, stop=True)
                nc.vector.tensor_copy(CTf32[:], psa[:])

                # (0.01 B)^T = VB @ (0.01 SB @ UB^T)
                p2 = wp.tile([N, N], F32, tag="us")
                nc.vector.tensor_scalar_mul(p2[:], ubt[:], sb01[:])
                psb = psp.tile([N, N], F32, tag="psB")
                nc.tensor.matmul(psb[:], vbt[:], p2[:], start=True, stop=True)
                nc.vector.tensor_copy(BpTm[:], psb[:])
                # untransposed 0.01 B = UB @ (0.01 SB @ VB^T)
                p2b = wp.tile([N, N], F32, tag="us")
                nc.vector.tensor_scalar_mul(p2b[:], vbt[:], sb01[:])
                psb2 = psp.tile([N, N], F32, tag="psB")
                nc.tensor.matmul(psb2[:], ubt[:], p2b[:], start=True, stop=True)
                bp_un = ep.tile([N, N], F32, tag="Bpun")
                nc.vector.tensor_copy(bp_un[:], psb2[:])

                # M = UA @ (sa05 * UA^T) = 0.5*UA SA UA^T (symmetric)
                p3 = wp.tile([N, N], F32, tag="negx")
                nc.vector.tensor_scalar_mul(p3[:], uat[:], sa05[:])
                psm = psp.tile([N, N], F32, tag="psA")
                nc.tensor.matmul(psm[:], uat[:], p3[:], start=True, stop=True)
                # YA = Uy - Uy^T; q2 = -0.005*YA
                uy = wp.tile([N, N], F32, tag="us")
                nc.vector.tensor_mul(uy[:], zt["ZA_Y"][:], masku[:])
                pst2 = psp.tile([N, N], F32, tag="psB")
                nc.tensor.transpose(pst2[:], uy[:], ident[:])
                nc.vector.tensor_scalar_mul(uy[:], uy[:], 0.5 * STEP)
                q2 = wp.tile([N, N], F32, tag="T")
                nc.vector.scalar_tensor_tensor(
                    q2[:], pst2[:], 0.5 * STEP, uy[:], op0=ALU.mult, op1=ALU.subtract
                )
                # (0.01 A)^T = 0.01*M + q2 ; untransposed 0.01 A = 0.01*M - q2
                nc.vector.scalar_tensor_tensor(
                    A01Tm[:], psm[:], STEP, q2[:], op0=ALU.mult, op1=ALU.add
                )
                a01_un = ep.tile([N, N], F32, tag="A01un")
                nc.vector.scalar_tensor_tensor(
                    a01_un[:], psm[:], STEP, q2[:], op0=ALU.mult, op1=ALU.subtract
                )

                # P^T = C^T + (0.01 A)^T C^T ;  Q^T = (0.01 B)^T C^T
                psw = psp.tile([N, N], F32, tag="psA")
                nc.tensor.matmul(psw[:], a01_un[:], CTf32[:], start=True, stop=True)
                nc.vector.scalar_tensor_tensor(
                    PTm[:], psw[:], 1.0, CTf32[:], op0=ALU.mult, op1=ALU.add
                )
                psq = psp.tile([N, N], F32, tag="psB")
                nc.tensor.matmul(psq[:], bp_un[:], CTf32[:], start=True, stop=True)
                nc.vector.tensor_copy(QTm[:], psq[:])

                # r = 0.01 C bx + by
                psr = psp.tile([N, 1], F32, tag="psA", name="psr")
                nc.tensor.matmul(psr[:], CTf32[:], bxp_c[:], start=True, stop=True)
                nc.vector.scalar_tensor_tensor(
                    r_c[:], psr[:], 1.0, by_c[:], op0=ALU.mult, op1=ALU.add
                )

            # ------- recurrence (split-carry) -------
            with (
                tc.tile_pool(name="xbuf", bufs=2) as xbufp,
                tc.tile_pool(name="stage", bufs=2) as stagep,
                tc.tile_pool(name="xr", bufs=3) as xrp,
                tc.tile_pool(name="th", bufs=3) as thp,
                tc.tile_pool(name="psy", bufs=3, space="PSUM") as psyp,
                tc.tile_pool(name="psx", bufs=3, space="PSUM") as psxp,
                tc.tile_pool(name="pstr", bufs=2, space="PSUM") as pstrp,
            ):
                # xbuf column slot for local step s: pairs (i, i+half) are
                # adjacent so the PE transpose reads one contiguous block
                # (walrus: matmul weight APs must have a single free dim).
                def slot(s):
                    return 2 * (s % half) + (s // half)

                # rounded copy of the state for matmul consumption
                xr_prev = xrp.tile([N, BSH], mdt, tag="xr", name="xr_init")
                nc.vector.tensor_copy(xr_prev[:], x0_c[:])
                # th_0 = tanh(C x_0 + by), fp32 matmul (one-time)
                psy0 = psyp.tile([N, BSH], F32, tag="psy", name="psy0")
                nc.tensor.matmul(psy0[:], CTf32[:], x0_c[:], start=True, stop=True)
                th_prev = thp.tile([N, BSH], mdt, tag="th", name="th_init")
                nc.scalar.activation(
                    th_prev[:], psy0[:], ACTF.Tanh, bias=by_c[:], scale=1.0
                )

                xb_prev = None
                for c in range(nchunks):
                    xb = xbufp.tile([N, tc_chunk * BSH], F32, tag="xb")
                    st = stagep.tile([128, half * N], F32, tag="st")
                    if c == 0:
                        nc.vector.tensor_copy(xb[:, 0:BSH], x0_c[:])
                    for s in range(tc_chunk):
                        t = c * tc_chunk + s
                        if t > 0:
                            if s > 0:
                                pxb, ps_ = xb, slot(s - 1)
                            else:
                                pxb, ps_ = xb_prev, slot(tc_chunk - 1)
                            xprev_f32 = pxb[:, ds(ps_ * BSH, BSH)]
                            # th-chain: y_t = P xr + Q th ; th_t = tanh(y+r)
                            psy = psyp.tile([N, BSH], F32, tag="psy")
                            nc.tensor.matmul(
                                psy[:], QTm[:], th_prev[:], start=True, stop=False
                            )
                            nc.tensor.matmul(
                                psy[:], PTm[:], xr_prev[:], start=False, stop=True
                            )
                            # x-chain: psx = 0.01A xr + 0.01B th
                            psx = psxp.tile([N, BSH], F32, tag="psx")
                            nc.tensor.matmul(
                                psx[:], BpTm[:], th_prev[:], start=True, stop=False
                            )
                            nc.tensor.matmul(
                                psx[:], A01Tm[:], xr_prev[:], start=False, stop=True
                            )
                            th_new = thp.tile([N, BSH], mdt, tag="th")
                            nc.scalar.activation(
                                th_new[:], psy[:], ACTF.Tanh, bias=r_c[:], scale=1.0
                            )
                            # chain-critical: rounded next state for the matmuls
                            xr_new = xrp.tile([N, BSH], mdt, tag="xr")
                            nc.vector.scalar_tensor_tensor(
                                xr_new[:], psx[:], bxp_c[:], xprev_f32,
                                op0=ALU.add, op1=ALU.add,
                            )
                            # exact fp32 state carry (also the output value)
                            nc.vector.scalar_tensor_tensor(
                                xb[:, ds(slot(s) * BSH, BSH)], psx[:], bxp_c[:],
                                xprev_f32, op0=ALU.add, op1=ALU.add,
                            )
                            xr_prev = xr_new
                            th_prev = th_new
                        if s >= half:
                            i = s - half
                            # transpose steps (i, i+half): adjacent slots
                            # (2i, 2i+1) -> one contiguous 128-col block
                            pstr = pstrp.tile([128, N], F32, tag="pstr")
                            nc.tensor.transpose(
                                pstr[:], xb[:, ds(2 * i * BSH, 2 * BSH)], ident[:]
                            )
                            nc.scalar.copy(st[:, ds(i * N, N)], pstr[:])
                    for h in range(2):
                        t0 = c * tc_chunk + h * half
                        dram_ap = out[:, t0:t0 + half, :].rearrange(
                            "b i n -> b (i n)"
                        )
                        nc.sync.dma_start(
                            out=dram_ap, in_=st[h * 64:(h + 1) * 64, :]
                        )
                    xb_prev = xb

    nc.compile()
    return nc


_CACHED = {}


def _get_program(tmax=TMAX, tc_chunk=32, mdt=BF16):
    key = (tmax, tc_chunk, str(mdt))
    if key not in _CACHED:
        _CACHED[key] = build_program(tmax, tc_chunk, mdt)
    return _CACHED[key]


def make_in_maps(inputs, tmax=TMAX):
    X0 = np.ascontiguousarray(np.asarray(inputs["X0"], dtype=np.float32))
    base = {
        name: np.ascontiguousarray(np.asarray(inputs[name], dtype=np.float32))
        for name in PARAM_NAMES
    }
    base["bx"] = np.ascontiguousarray(
        np.asarray(inputs["bx"], dtype=np.float32).reshape(N, 1)
    )
    base["by"] = np.ascontiguousarray(
        np.asarray(inputs["by"], dtype=np.float32).reshape(N, 1)
    )
    in_maps = []
    for c in range(NCORES):
        m = dict(base)
        m["x0"] = np.ascontiguousarray(X0[c * BSH:(c + 1) * BSH].T)
        in_maps.append(m)
    return in_maps


def run_spmd(inputs, tmax=TMAX, tc_chunk=32, trace=False, tmpdir=None, mdt=BF16):
    nc = _get_program(tmax, tc_chunk, mdt)
    in_maps = make_in_maps(inputs, tmax)
    res = run_bass_kernel_spmd(
        nc, in_maps, list(range(NCORES)), trace=trace, tmpdir=tmpdir
    )
    outs = [res.results[c]["out"] for c in range(NCORES)]
    full = np.concatenate(outs, axis=0)
    return full, res


def kernel(**inputs):
    full, _ = run_spmd(inputs)
    return full


# revision 13
# speedup vs baseline: 3.0137x; 1.0238x over previous
"""LurieNet-k Trainium2 kernel.

Computes, from the raw parametrization tensors, the matrices
  C = UC @ SC @ VC^T,  B = UB @ SB @ VB^T,
  A = 0.5*UA @ SA @ UA^T + 0.5*YA  (SA = -(alpha_upp*I + GA))
entirely on device (matrix exponentials of skew matrices via
scaling-and-squaring Taylor), then runs the 511-step recurrence
  y  = C x + by
  x' = x + (0.01*A x + 0.01*B tanh(y) + 0.01*bx)
on a (128, 64) state shard per NeuronCore (batch data-parallel over the
8 cores), writing the full (b, t, n) trajectory.

Precision: the system amplifies per-step state perturbations, so the
identity part of the state update is carried in fp32 OUTSIDE the
matmuls (split-carry): the matmuls only apply gains << 1 (C ~ 0.3,
0.01A ~ 0.003, 0.01B ~ 0.003), so their operands can be bf16 without
error blowup (measured ~2e-3 final rel err vs fp32's 6e-6; naive bf16
with A' = I + 0.01A as a matmul diverges to 0.36). bf16 weights get
FWL weight loads, which matters because the kernel is otherwise
LDWEIGHTS-throughput bound.
"""

import sys

for _p in ("/opt/trn_rl_repo",):
    if _p not in sys.path:
        sys.path.insert(0, _p)

import numpy as np

import concourse.bass as bass
import concourse.mybir as mybir
import concourse.tile as tile
from concourse import bacc
from concourse import bass_isa
from concourse.bass import ds
from concourse.bass_utils import run_bass_kernel_spmd
from concourse.masks import make_identity, make_upper_triangular

F32 = mybir.dt.float32
BF16 = mybir.dt.bfloat16
ALU = mybir.AluOpType
ACTF = mybir.ActivationFunctionType
AXIS = mybir.AxisListType

N = 128          # state dim
TMAX = 512       # time steps (including t=0)
BS = 512         # global batch
NCORES = 8
BSH = BS // NCORES   # 64 batch columns per core
STEP = 0.01
KTOP = 4

EXPM_SCAL = 6    # expm scaling: X = S / 2**EXPM_SCAL, then 6 squarings
EXPM_TERMS = 7   # Taylor terms in the Horner evaluation

PARAM_NAMES = [
    "ZA_Y", "ZA_U", "ZA_G", "ZB_U", "ZB_V", "ZB_S", "ZC_U", "ZC_V", "ZC_S",
]


def build_program(tmax=TMAX, tc_chunk=32, mdt=BF16):
    """Build the single-NeuronCore Bass program (run SPMD on all 8 cores).

    mdt: dtype of the recurrence matmul operands (weights, rounded state
    copy, tanh output). The fp32 state carry is exact regardless.
    """
    assert tmax % tc_chunk == 0 and tc_chunk % 2 == 0
    half = tc_chunk // 2
    nchunks = tmax // tc_chunk

    nc = bacc.Bacc(
        "TRN2",
        target_bir_lowering=False,
        debug=False,
        enable_asserts=False,
        num_devices=NCORES,
    )

    x0 = nc.dram_tensor("x0", [N, BSH], F32, kind="ExternalInput")
    zs = {
        name: nc.dram_tensor(name, [N, N], F32, kind="ExternalInput")
        for name in PARAM_NAMES
    }
    bx_d = nc.dram_tensor("bx", [N, 1], F32, kind="ExternalInput")
    by_d = nc.dram_tensor("by", [N, 1], F32, kind="ExternalInput")
    out = nc.dram_tensor("out", [BSH, tmax, N], F32, kind="ExternalOutput")

    with tile.TileContext(nc) as tc:
        with tc.tile_pool(name="const", bufs=1) as constp:
            ident = constp.tile([N, N], F32, tag="ident")
            make_identity(nc, ident[:])
            masku = constp.tile([N, N], F32, tag="masku")
            make_upper_triangular(nc, masku[:], val=1.0, diag=False)

            by_c = constp.tile([N, 1], F32, tag="by")
            nc.sync.dma_start(out=by_c[:], in_=by_d[:])
            bx_c = constp.tile([N, 1], F32, tag="bxraw")
            nc.sync.dma_start(out=bx_c[:], in_=bx_d[:])
            bxp_c = constp.tile([N, 1], F32, tag="bxp")
            nc.vector.tensor_scalar_mul(bxp_c[:], bx_c[:], STEP)
            x0_c = constp.tile([N, BSH], F32, tag="x0c")
            nc.sync.dma_start(out=x0_c[:], in_=x0[:])

            # ------- setup phase: expm's + weight assembly -------
            # Recurrence uses the P-form to split the serial chain in two:
            #   y_t  = P x_{t-1} + Q th_{t-1} + r,  th_t = tanh(y_t)
            #   x_t  = x_{t-1} + (0.01A) x_{t-1} + (0.01B) th_{t-1} + 0.01 bx
            # with P = C + 0.01 C A, Q = 0.01 C B, r = 0.01 C bx + by.
            PTm = constp.tile([N, N], mdt, tag="PTm")      # P^T
            QTm = constp.tile([N, N], mdt, tag="QTm")      # Q^T
            A01Tm = constp.tile([N, N], mdt, tag="A01Tm")  # (0.01 A)^T
            BpTm = constp.tile([N, N], mdt, tag="BpTm")    # (0.01 B)^T
            CTf32 = constp.tile([N, N], F32, tag="CTf32")  # C^T fp32
            r_c = constp.tile([N, 1], F32, tag="rc")       # 0.01 C bx + by

            with (
                tc.tile_pool(name="zbuf", bufs=1) as zp,
                tc.tile_pool(name="work", bufs=2) as wp,
                tc.tile_pool(name="eres", bufs=1) as ep,
                tc.tile_pool(name="small", bufs=1) as sp,
                tc.tile_pool(name="pss", bufs=2, space="PSUM") as psp,
            ):
                zt = {}
                for name in PARAM_NAMES:
                    zt[name] = zp.tile([N, N], F32, tag=name, name=f"z_{name}")
                    nc.sync.dma_start(out=zt[name][:], in_=zs[name][:])

                def expm_transposed(z_tile, tag):
                    """Return expm(skew(Z))^T for the strict-upper skew of Z.

                    Maintains the (T, T^T) pair through Horner + squaring so
                    no PE transposes are needed: with negX = X^T = -X,
                      X @ T     = matmul(lhsT=negX, rhs=T)
                      T^T @ X^T = matmul(lhsT=T,    rhs=negX)
                    """
                    scal = 1.0 / (2.0 ** EXPM_SCAL)
                    us = wp.tile([N, N], F32, tag="us")
                    # us = (Z * scal) * mask  (strict upper part, prescaled)
                    nc.vector.scalar_tensor_tensor(
                        us[:], z_tile[:], scal, masku[:], op0=ALU.mult, op1=ALU.mult
                    )
                    pst = psp.tile([N, N], F32, tag="psA")
                    nc.tensor.transpose(pst[:], us[:], ident[:])
                    negx = wp.tile([N, N], F32, tag="negx")
                    # negX = us^T - us  ( = X^T = -X for X = us - us^T )
                    nc.vector.scalar_tensor_tensor(
                        negx[:], pst[:], 1.0, us[:], op0=ALU.mult, op1=ALU.subtract
                    )

                    t_cur, tt_cur = ident, ident
                    for j in range(EXPM_TERMS, 0, -1):
                        psa = psp.tile([N, N], F32, tag="psA")
                        psb = psp.tile([N, N], F32, tag="psB")
                        nc.tensor.matmul(
                            psa[:], negx[:], t_cur[:], start=True, stop=True
                        )
                        nc.tensor.matmul(
                            psb[:], t_cur[:], negx[:], start=True, stop=True
                        )
                        t_new = wp.tile([N, N], F32, tag="T")
                        tt_new = wp.tile([N, N], F32, tag="TT")
                        nc.vector.scalar_tensor_tensor(
                            t_new[:], psa[:], 1.0 / j, ident[:],
                            op0=ALU.mult, op1=ALU.add,
                        )
                        nc.vector.scalar_tensor_tensor(
                            tt_new[:], psb[:], 1.0 / j, ident[:],
                            op0=ALU.mult, op1=ALU.add,
                        )
                        t_cur, tt_cur = t_new, tt_new
                    for _ in range(EXPM_SCAL):
                        psa = psp.tile([N, N], F32, tag="psA")
                        psb = psp.tile([N, N], F32, tag="psB")
                        nc.tensor.matmul(
                            psa[:], tt_cur[:], t_cur[:], start=True, stop=True
                        )
                        nc.tensor.matmul(
                            psb[:], t_cur[:], tt_cur[:], start=True, stop=True
                        )
                        t_new = wp.tile([N, N], F32, tag="T")
                        tt_new = wp.tile([N, N], F32, tag="TT")
                        nc.vector.tensor_copy(t_new[:], psa[:])
                        nc.vector.tensor_copy(tt_new[:], psb[:])
                        t_cur, tt_cur = t_new, tt_new
                    res = ep.tile([N, N], F32, tag=tag)
                    nc.vector.tensor_copy(res[:], tt_cur[:])
                    return res

                uct = expm_transposed(zt["ZC_U"], "UCT")   # UC^T
                vct = expm_transposed(zt["ZC_V"], "VCT")   # VC^T
                ubt = expm_transposed(zt["ZB_U"], "UBT")   # UB^T
                vbt = expm_transposed(zt["ZB_V"], "VBT")   # VB^T
                uat = expm_transposed(zt["ZA_U"], "UAT")   # UA^T

                def absdiag_col(z_tile, tag):
                    tmp = wp.tile([N, N], F32, tag="us")
                    nc.vector.tensor_mul(tmp[:], z_tile[:], ident[:])
                    col = sp.tile([N, 1], F32, tag=tag, name=f"col_{tag}")
                    nc.vector.tensor_reduce(
                        col[:], tmp[:], AXIS.X, ALU.add, apply_absolute_value=True
                    )
                    return col

                dc_col = absdiag_col(zt["ZC_S"], "dc")   # |diag(ZC_S)|
                db_col = absdiag_col(zt["ZB_S"], "db")   # |diag(ZB_S)|
                ga_col = absdiag_col(zt["ZA_G"], "ga")   # |diag(ZA_G)|

                # top-4: alpha = sqrt(sum_i (b_i c_i)^2), b/c sorted desc.
                bwork = sp.tile([N, 1], F32, tag="bwork")
                cwork = sp.tile([N, 1], F32, tag="cwork")
                nc.vector.tensor_copy(bwork[:], db_col[:])
                nc.vector.tensor_copy(cwork[:], dc_col[:])
                acc = sp.tile([N, 1], F32, tag="acc")
                nc.vector.memset(acc[:], 0.0)
                bmax = sp.tile([N, 1], F32, tag="bmax")
                cmax = sp.tile([N, 1], F32, tag="cmax")
                prod = sp.tile([N, 1], F32, tag="prod")
                gmask = sp.tile([N, 1], F32, tag="gmask")
                tdrop = sp.tile([N, 1], F32, tag="tdrop")
                for i in range(KTOP):
                    nc.gpsimd.partition_all_reduce(
                        bmax[:], bwork[:], N, bass_isa.ReduceOp.max
                    )
                    nc.gpsimd.partition_all_reduce(
                        cmax[:], cwork[:], N, bass_isa.ReduceOp.max
                    )
                    nc.vector.tensor_mul(prod[:], bmax[:], cmax[:])
                    nc.vector.tensor_mul(prod[:], prod[:], prod[:])
                    nc.vector.tensor_add(acc[:], acc[:], prod[:])
                    if i < KTOP - 1:
                        # zero out the extracted max (values all > 0)
                        nc.vector.tensor_single_scalar(
                            gmask[:], bwork[:], bmax[:], ALU.is_ge
                        )
                        nc.vector.tensor_mul(tdrop[:], bwork[:], gmask[:])
                        nc.vector.tensor_sub(bwork[:], bwork[:], tdrop[:])
                        nc.vector.tensor_single_scalar(
                            gmask[:], cwork[:], cmax[:], ALU.is_ge
                        )
                        nc.vector.tensor_mul(tdrop[:], cwork[:], gmask[:])
                        nc.vector.tensor_sub(cwork[:], cwork[:], tdrop[:])
                alpha = sp.tile([N, 1], F32, tag="alpha")
                nc.scalar.activation(alpha[:], acc[:], ACTF.Sqrt)

                # sa05 = -0.5*(alpha + gA)  (per-partition row scale of UA^T)
                sa05 = sp.tile([N, 1], F32, tag="sa05")
                nc.vector.tensor_scalar(
                    sa05[:], ga_col[:], alpha[:], -0.5, op0=ALU.add, op1=ALU.mult
                )
                sb01 = sp.tile([N, 1], F32, tag="sb01")
                nc.vector.tensor_scalar_mul(sb01[:], db_col[:], STEP)

                # C^T = VC @ (SC @ UC^T)
                p1 = wp.tile([N, N], F32, tag="us")
                nc.vector.tensor_scalar_mul(p1[:], uct[:], dc_col[:])
                psa = psp.tile([N, N], F32, tag="psA")
                nc.tensor.matmul(psa[:], vct[:], p1[:], start=True, stop=True)
                nc.vector.tensor_copy(CTf32[:], psa[:])

                # (0.01 B)^T = VB @ (0.01 SB @ UB^T)
                p2 = wp.tile([N, N], F32, tag="us")
                nc.vector.tensor_scalar_mul(p2[:], ubt[:], sb01[:])
                psb = psp.tile([N, N], F32, tag="psB")
                nc.tensor.matmul(psb[:], vbt[:], p2[:], start=True, stop=True)
                nc.vector.tensor_copy(BpTm[:], psb[:])
                # untransposed 0.01 B = UB @ (0.01 SB @ VB^T)
                p2b = wp.tile([N, N], F32, tag="us")
                nc.vector.tensor_scalar_mul(p2b[:], vbt[:], sb01[:])
                psb2 = psp.tile([N, N], F32, tag="psB")
                nc.tensor.matmul(psb2[:], ubt[:], p2b[:], start=True, stop=True)
                bp_un = ep.tile([N, N], F32, tag="Bpun")
                nc.vector.tensor_copy(bp_un[:], psb2[:])

                # M = UA @ (sa05 * UA^T) = 0.5*UA SA UA^T (symmetric)
                p3 = wp.tile([N, N], F32, tag="negx")
                nc.vector.tensor_scalar_mul(p3[:], uat[:], sa05[:])
                psm = psp.tile([N, N], F32, tag="psA")
                nc.tensor.matmul(psm[:], uat[:], p3[:], start=True, stop=True)
                # YA = Uy - Uy^T; q2 = -0.005*YA
                uy = wp.tile([N, N], F32, tag="us")
                nc.vector.tensor_mul(uy[:], zt["ZA_Y"][:], masku[:])
                pst2 = psp.tile([N, N], F32, tag="psB")
                nc.tensor.transpose(pst2[:], uy[:], ident[:])
                nc.vector.tensor_scalar_mul(uy[:], uy[:], 0.5 * STEP)
                q2 = wp.tile([N, N], F32, tag="T")
                nc.vector.scalar_tensor_tensor(
                    q2[:], pst2[:], 0.5 * STEP, uy[:], op0=ALU.mult, op1=ALU.subtract
                )
                # (0.01 A)^T = 0.01*M + q2 ; untransposed 0.01 A = 0.01*M - q2
                nc.vector.scalar_tensor_tensor(
                    A01Tm[:], psm[:], STEP, q2[:], op0=ALU.mult, op1=ALU.add
                )
                a01_un = ep.tile([N, N], F32, tag="A01un")
                nc.vector.scalar_tensor_tensor(
                    a01_un[:], psm[:], STEP, q2[:], op0=ALU.mult, op1=ALU.subtract
                )

                # P^T = C^T + (0.01 A)^T C^T ;  Q^T = (0.01 B)^T C^T
                psw = psp.tile([N, N], F32, tag="psA")
                nc.tensor.matmul(psw[:], a01_un[:], CTf32[:], start=True, stop=True)
                nc.vector.scalar_tensor_tensor(
                    PTm[:], psw[:], 1.0, CTf32[:], op0=ALU.mult, op1=ALU.add
                )
                psq = psp.tile([N, N], F32, tag="psB")
                nc.tensor.matmul(psq[:], bp_un[:], CTf32[:], start=True, stop=True)
                nc.vector.tensor_copy(QTm[:], psq[:])

                # r = 0.01 C bx + by
                psr = psp.tile([N, 1], F32, tag="psA", name="psr")
                nc.tensor.matmul(psr[:], CTf32[:], bxp_c[:], start=True, stop=True)
                nc.vector.scalar_tensor_tensor(
                    r_c[:], psr[:], 1.0, by_c[:], op0=ALU.mult, op1=ALU.add
                )

            # ------- recurrence (split-carry) -------
            with (
                tc.tile_pool(name="xbuf", bufs=2) as xbufp,
                tc.tile_pool(name="stage", bufs=2) as stagep,
                tc.tile_pool(name="xr", bufs=3) as xrp,
                tc.tile_pool(name="th", bufs=3) as thp,
                tc.tile_pool(name="psy", bufs=3, space="PSUM") as psyp,
                tc.tile_pool(name="psx", bufs=3, space="PSUM") as psxp,
                tc.tile_pool(name="pstr", bufs=2, space="PSUM") as pstrp,
            ):
                # xbuf column slot for local step s: pairs (i, i+half) are
                # adjacent so the PE transpose reads one contiguous block
                # (walrus: matmul weight APs must have a single free dim).
                def slot(s):
                    return 2 * (s % half) + (s // half)

                # rounded copy of the state for matmul consumption
                xr_prev = xrp.tile([N, BSH], mdt, tag="xr", name="xr_init")
                nc.vector.tensor_copy(xr_prev[:], x0_c[:])
                # th_0 = tanh(C x_0 + by), fp32 matmul (one-time)
                psy0 = psyp.tile([N, BSH], F32, tag="psy", name="psy0")
                nc.tensor.matmul(psy0[:], CTf32[:], x0_c[:], start=True, stop=True)
                th_prev = thp.tile([N, BSH], mdt, tag="th", name="th_init")
                nc.scalar.activation(
                    th_prev[:], psy0[:], ACTF.Tanh, bias=by_c[:], scale=1.0
                )

                xb_prev = None
                for c in range(nchunks):
                    xb = xbufp.tile([N, tc_chunk * BSH], F32, tag="xb")
                    st = stagep.tile([128, half * N], F32, tag="st")
                    if c == 0:
                        nc.vector.tensor_copy(xb[:, 0:BSH], x0_c[:])
                    for s in range(tc_chunk):
                        t = c * tc_chunk + s
                        if t > 0:
                            if s > 0:
                                pxb, ps_ = xb, slot(s - 1)
                            else:
                                pxb, ps_ = xb_prev, slot(tc_chunk - 1)
                            xprev_f32 = pxb[:, ds(ps_ * BSH, BSH)]
                            # th-chain: y_t = P xr + Q th ; th_t = tanh(y+r)
                            psy = psyp.tile([N, BSH], F32, tag="psy")
                            nc.tensor.matmul(
                                psy[:], QTm[:], th_prev[:], start=True, stop=False
                            )
                            nc.tensor.matmul(
                                psy[:], PTm[:], xr_prev[:], start=False, stop=True
                            )
                            # x-chain: psx = 0.01A xr + 0.01B th
                            psx = psxp.tile([N, BSH], F32, tag="psx")
                            nc.tensor.matmul(
                                psx[:], BpTm[:], th_prev[:], start=True, stop=False
                            )
                            nc.tensor.matmul(
                                psx[:], A01Tm[:], xr_prev[:], start=False, stop=True
                            )
                            th_new = thp.tile([N, BSH], mdt, tag="th")
                            nc.scalar.activation(
                                th_new[:], psy[:], ACTF.Tanh, bias=r_c[:], scale=1.0
                            )
                            # chain-critical: rounded next state for the matmuls
                            xr_new = xrp.tile([N, BSH], mdt, tag="xr")
                            nc.vector.scalar_tensor_tensor(
                                xr_new[:], psx[:], bxp_c[:], xprev_f32,
                                op0=ALU.add, op1=ALU.add,
                            )
                            # exact fp32 state carry (also the output value)
                            nc.vector.scalar_tensor_tensor(
                                xb[:, ds(slot(s) * BSH, BSH)], psx[:], bxp_c[:],
                                xprev_f32, op0=ALU.add, op1=ALU.add,
                            )
                            xr_prev = xr_new
                            th_prev = th_new
                        if s >= half:
                            i = s - half
                            # transpose steps (i, i+half): adjacent slots
                            # (2i, 2i+1) -> one contiguous 128-col block.
                            # Two pair-transposes share one psum tile; a
                            # single ACT copy drains both (fewer ACT ops on
                            # the engine the chain-critical tanh runs on).
                            if i % 2 == 0:
                                pstr = pstrp.tile([128, 2 * N], F32, tag="pstr")
                            nc.tensor.transpose(
                                pstr[:, ds((i % 2) * N, N)],
                                xb[:, ds(2 * i * BSH, 2 * BSH)],
                                ident[:],
                            )
                            if i % 2 == 1 or s == tc_chunk - 1:
                                lo = (i - (i % 2)) * N
                                width = (i % 2 + 1) * N
                                nc.scalar.copy(
                                    st[:, ds(lo, width)], pstr[:, 0:width]
                                )
                    for h in range(2):
                        t0 = c * tc_chunk + h * half
                        dram_ap = out[:, t0:t0 + half, :].rearrange(
                            "b i n -> b (i n)"
                        )
                        nc.sync.dma_start(
                            out=dram_ap, in_=st[h * 64:(h + 1) * 64, :]
                        )
                    xb_prev = xb

    nc.compile()
    return nc


_CACHED = {}


def _get_program(tmax=TMAX, tc_chunk=32, mdt=BF16):
    key = (tmax, tc_chunk, str(mdt))
    if key not in _CACHED:
        _CACHED[key] = build_program(tmax, tc_chunk, mdt)
    return _CACHED[key]


def make_in_maps(inputs, tmax=TMAX):
    X0 = np.ascontiguousarray(np.asarray(inputs["X0"], dtype=np.float32))
    base = {
        name: np.ascontiguousarray(np.asarray(inputs[name], dtype=np.float32))
        for name in PARAM_NAMES
    }
    base["bx"] = np.ascontiguousarray(
        np.asarray(inputs["bx"], dtype=np.float32).reshape(N, 1)
    )
    base["by"] = np.ascontiguousarray(
        np.asarray(inputs["by"], dtype=np.float32).reshape(N, 1)
    )
    in_maps = []
    for c in range(NCORES):
        m = dict(base)
        m["x0"] = np.ascontiguousarray(X0[c * BSH:(c + 1) * BSH].T)
        in_maps.append(m)
    return in_maps


def run_spmd(inputs, tmax=TMAX, tc_chunk=32, trace=False, tmpdir=None, mdt=BF16):
    nc = _get_program(tmax, tc_chunk, mdt)
    in_maps = make_in_maps(inputs, tmax)
    res = run_bass_kernel_spmd(
        nc, in_maps, list(range(NCORES)), trace=trace, tmpdir=tmpdir
    )
    outs = [res.results[c]["out"] for c in range(NCORES)]
    full = np.concatenate(outs, axis=0)
    return full, res


def kernel(**inputs):
    full, _ = run_spmd(inputs)
    return full


# revision 14
# speedup vs baseline: 3.1430x; 1.0429x over previous
"""LurieNet-k Trainium2 kernel.

Computes, from the raw parametrization tensors, the matrices
  C = UC @ SC @ VC^T,  B = UB @ SB @ VB^T,
  A = 0.5*UA @ SA @ UA^T + 0.5*YA  (SA = -(alpha_upp*I + GA))
entirely on device (matrix exponentials of skew matrices via
scaling-and-squaring Taylor), then runs the 511-step recurrence
  y  = C x + by
  x' = x + (0.01*A x + 0.01*B tanh(y) + 0.01*bx)
on a (128, 64) state shard per NeuronCore (batch data-parallel over the
8 cores), writing the full (b, t, n) trajectory.

Precision: the system amplifies per-step state perturbations, so the
identity part of the state update is carried in fp32 OUTSIDE the
matmuls (split-carry): the matmuls only apply gains << 1 (C ~ 0.3,
0.01A ~ 0.003, 0.01B ~ 0.003), so their operands can be bf16 without
error blowup (measured ~2e-3 final rel err vs fp32's 6e-6; naive bf16
with A' = I + 0.01A as a matmul diverges to 0.36). bf16 weights get
FWL weight loads, which matters because the kernel is otherwise
LDWEIGHTS-throughput bound.
"""

import sys

for _p in ("/opt/trn_rl_repo",):
    if _p not in sys.path:
        sys.path.insert(0, _p)

import numpy as np

import concourse.bass as bass
import concourse.mybir as mybir
import concourse.tile as tile
from concourse import bacc
from concourse import bass_isa
from concourse.bass import ds
from concourse.bass_utils import run_bass_kernel_spmd
from concourse.masks import make_identity, make_upper_triangular

F32 = mybir.dt.float32
F32R = mybir.dt.float32r
BF16 = mybir.dt.bfloat16
ALU = mybir.AluOpType
ACTF = mybir.ActivationFunctionType
AXIS = mybir.AxisListType

N = 128          # state dim
TMAX = 512       # time steps (including t=0)
BS = 512         # global batch
NCORES = 8
BSH = BS // NCORES   # 64 batch columns per core
STEP = 0.01
KTOP = 4

EXPM_SCAL = 5    # expm scaling: X = S / 2**EXPM_SCAL, then 5 squarings
EXPM_TERMS = 6   # Taylor terms in the Horner evaluation

PARAM_NAMES = [
    "ZA_Y", "ZA_U", "ZA_G", "ZB_U", "ZB_V", "ZB_S", "ZC_U", "ZC_V", "ZC_S",
]


def build_program(tmax=TMAX, tc_chunk=32, mdt=BF16):
    """Build the single-NeuronCore Bass program (run SPMD on all 8 cores).

    mdt: dtype of the recurrence matmul operands (weights, rounded state
    copy, tanh output). The fp32 state carry is exact regardless.
    """
    assert tmax % tc_chunk == 0 and tc_chunk % 2 == 0
    half = tc_chunk // 2
    nchunks = tmax // tc_chunk

    nc = bacc.Bacc(
        "TRN2",
        target_bir_lowering=False,
        debug=False,
        enable_asserts=False,
        num_devices=NCORES,
    )

    x0 = nc.dram_tensor("x0", [N, BSH], F32, kind="ExternalInput")
    zs = {
        name: nc.dram_tensor(name, [N, N], F32, kind="ExternalInput")
        for name in PARAM_NAMES
    }
    bx_d = nc.dram_tensor("bx", [N, 1], F32, kind="ExternalInput")
    by_d = nc.dram_tensor("by", [N, 1], F32, kind="ExternalInput")
    out = nc.dram_tensor("out", [BSH, tmax, N], F32, kind="ExternalOutput")

    with tile.TileContext(nc) as tc:
        with tc.tile_pool(name="const", bufs=1) as constp:
            ident = constp.tile([N, N], F32, tag="ident")
            make_identity(nc, ident[:])
            masku = constp.tile([N, N], F32, tag="masku")
            make_upper_triangular(nc, masku[:], val=1.0, diag=False)
            ident_r32 = constp.tile([N, N], F32R, tag="ident_r32")
            nc.vector.tensor_copy(ident_r32[:], ident[:])

            by_c = constp.tile([N, 1], F32, tag="by")
            nc.sync.dma_start(out=by_c[:], in_=by_d[:])
            bx_c = constp.tile([N, 1], F32, tag="bxraw")
            nc.sync.dma_start(out=bx_c[:], in_=bx_d[:])
            bxp_c = constp.tile([N, 1], F32, tag="bxp")
            nc.vector.tensor_scalar_mul(bxp_c[:], bx_c[:], STEP)
            x0_c = constp.tile([N, BSH], F32, tag="x0c")
            nc.sync.dma_start(out=x0_c[:], in_=x0[:])

            # ------- setup phase: expm's + weight assembly -------
            # Recurrence uses the P-form to split the serial chain in two:
            #   y_t  = P x_{t-1} + Q th_{t-1} + r,  th_t = tanh(y_t)
            #   x_t  = x_{t-1} + (0.01A) x_{t-1} + (0.01B) th_{t-1} + 0.01 bx
            # with P = C + 0.01 C A, Q = 0.01 C B, r = 0.01 C bx + by.
            PTm = constp.tile([N, N], mdt, tag="PTm")      # P^T
            QTm = constp.tile([N, N], mdt, tag="QTm")      # Q^T
            A01Tm = constp.tile([N, N], mdt, tag="A01Tm")  # (0.01 A)^T
            BpTm = constp.tile([N, N], mdt, tag="BpTm")    # (0.01 B)^T
            CTf32 = constp.tile([N, N], F32, tag="CTf32")  # C^T fp32
            r_c = constp.tile([N, 1], F32, tag="rc")       # 0.01 C bx + by

            with (
                tc.tile_pool(name="zbuf", bufs=1) as zp,
                tc.tile_pool(name="work", bufs=2) as wp,
                tc.tile_pool(name="eres", bufs=1) as ep,
                tc.tile_pool(name="small", bufs=1) as sp,
                tc.tile_pool(name="pss", bufs=2, space="PSUM") as psp,
            ):
                zt = {}
                for name in PARAM_NAMES:
                    zt[name] = zp.tile([N, N], F32, tag=name, name=f"z_{name}")
                    nc.sync.dma_start(out=zt[name][:], in_=zs[name][:])

                def expm_transposed(z_tile, tag):
                    """Return expm(skew(Z))^T for the strict-upper skew of Z.

                    Maintains the (T, T^T) pair through Horner + squaring so
                    no PE transposes are needed: with negX = X^T = -X,
                      X @ T     = matmul(lhsT=negX, rhs=T)
                      T^T @ X^T = matmul(lhsT=T,    rhs=negX)
                    """
                    scal = 1.0 / (2.0 ** EXPM_SCAL)
                    us = wp.tile([N, N], F32R, tag="us_r", name="us_r")
                    # us = (Z * scal) * mask  (strict upper part, prescaled)
                    nc.vector.scalar_tensor_tensor(
                        us[:], z_tile[:], scal, masku[:], op0=ALU.mult, op1=ALU.mult
                    )
                    pst = psp.tile([N, N], F32R, tag="psA", name="pst_r")
                    nc.tensor.transpose(pst[:], us[:], ident_r32[:])
                    negx = wp.tile([N, N], F32R, tag="negx")
                    # negX = us^T - us  ( = X^T = -X for X = us - us^T )
                    nc.vector.scalar_tensor_tensor(
                        negx[:], pst[:], 1.0, us[:], op0=ALU.mult, op1=ALU.subtract
                    )

                    t_cur, tt_cur = ident_r32, ident_r32
                    for j in range(EXPM_TERMS, 0, -1):
                        psa = psp.tile([N, N], F32, tag="psA")
                        psb = psp.tile([N, N], F32, tag="psB")
                        nc.tensor.matmul(
                            psa[:], negx[:], t_cur[:], start=True, stop=True
                        )
                        nc.tensor.matmul(
                            psb[:], t_cur[:], negx[:], start=True, stop=True
                        )
                        t_new = wp.tile([N, N], F32R, tag="T")
                        tt_new = wp.tile([N, N], F32R, tag="TT")
                        nc.vector.scalar_tensor_tensor(
                            t_new[:], psa[:], 1.0 / j, ident_r32[:],
                            op0=ALU.mult, op1=ALU.add,
                        )
                        nc.vector.scalar_tensor_tensor(
                            tt_new[:], psb[:], 1.0 / j, ident_r32[:],
                            op0=ALU.mult, op1=ALU.add,
                        )
                        t_cur, tt_cur = t_new, tt_new
                    for _ in range(EXPM_SCAL):
                        psa = psp.tile([N, N], F32, tag="psA")
                        psb = psp.tile([N, N], F32, tag="psB")
                        nc.tensor.matmul(
                            psa[:], tt_cur[:], t_cur[:], start=True, stop=True
                        )
                        nc.tensor.matmul(
                            psb[:], t_cur[:], tt_cur[:], start=True, stop=True
                        )
                        t_new = wp.tile([N, N], F32R, tag="T")
                        tt_new = wp.tile([N, N], F32R, tag="TT")
                        nc.vector.tensor_copy(t_new[:], psa[:])
                        nc.vector.tensor_copy(tt_new[:], psb[:])
                        t_cur, tt_cur = t_new, tt_new
                    res = ep.tile([N, N], F32, tag=tag)
                    nc.vector.tensor_copy(res[:], tt_cur[:])
                    return res

                uct = expm_transposed(zt["ZC_U"], "UCT")   # UC^T
                vct = expm_transposed(zt["ZC_V"], "VCT")   # VC^T
                ubt = expm_transposed(zt["ZB_U"], "UBT")   # UB^T
                vbt = expm_transposed(zt["ZB_V"], "VBT")   # VB^T
                uat = expm_transposed(zt["ZA_U"], "UAT")   # UA^T

                def absdiag_col(z_tile, tag):
                    tmp = wp.tile([N, N], F32, tag="us")
                    nc.vector.tensor_mul(tmp[:], z_tile[:], ident[:])
                    col = sp.tile([N, 1], F32, tag=tag, name=f"col_{tag}")
                    nc.vector.tensor_reduce(
                        col[:], tmp[:], AXIS.X, ALU.add, apply_absolute_value=True
                    )
                    return col

                dc_col = absdiag_col(zt["ZC_S"], "dc")   # |diag(ZC_S)|
                db_col = absdiag_col(zt["ZB_S"], "db")   # |diag(ZB_S)|
                ga_col = absdiag_col(zt["ZA_G"], "ga")   # |diag(ZA_G)|

                # top-4: alpha = sqrt(sum_i (b_i c_i)^2), b/c sorted desc.
                bwork = sp.tile([N, 1], F32, tag="bwork")
                cwork = sp.tile([N, 1], F32, tag="cwork")
                nc.vector.tensor_copy(bwork[:], db_col[:])
                nc.vector.tensor_copy(cwork[:], dc_col[:])
                acc = sp.tile([N, 1], F32, tag="acc")
                nc.vector.memset(acc[:], 0.0)
                bmax = sp.tile([N, 1], F32, tag="bmax")
                cmax = sp.tile([N, 1], F32, tag="cmax")
                prod = sp.tile([N, 1], F32, tag="prod")
                gmask = sp.tile([N, 1], F32, tag="gmask")
                tdrop = sp.tile([N, 1], F32, tag="tdrop")
                for i in range(KTOP):
                    nc.gpsimd.partition_all_reduce(
                        bmax[:], bwork[:], N, bass_isa.ReduceOp.max
                    )
                    nc.gpsimd.partition_all_reduce(
                        cmax[:], cwork[:], N, bass_isa.ReduceOp.max
                    )
                    nc.vector.tensor_mul(prod[:], bmax[:], cmax[:])
                    nc.vector.tensor_mul(prod[:], prod[:], prod[:])
                    nc.vector.tensor_add(acc[:], acc[:], prod[:])
                    if i < KTOP - 1:
                        # zero out the extracted max (values all > 0)
                        nc.vector.tensor_single_scalar(
                            gmask[:], bwork[:], bmax[:], ALU.is_ge
                        )
                        nc.vector.tensor_mul(tdrop[:], bwork[:], gmask[:])
                        nc.vector.tensor_sub(bwork[:], bwork[:], tdrop[:])
                        nc.vector.tensor_single_scalar(
                            gmask[:], cwork[:], cmax[:], ALU.is_ge
                        )
                        nc.vector.tensor_mul(tdrop[:], cwork[:], gmask[:])
                        nc.vector.tensor_sub(cwork[:], cwork[:], tdrop[:])
                alpha = sp.tile([N, 1], F32, tag="alpha")
                nc.scalar.activation(alpha[:], acc[:], ACTF.Sqrt)

                # sa05 = -0.5*(alpha + gA)  (per-partition row scale of UA^T)
                sa05 = sp.tile([N, 1], F32, tag="sa05")
                nc.vector.tensor_scalar(
                    sa05[:], ga_col[:], alpha[:], -0.5, op0=ALU.add, op1=ALU.mult
                )
                sb01 = sp.tile([N, 1], F32, tag="sb01")
                nc.vector.tensor_scalar_mul(sb01[:], db_col[:], STEP)

                # C^T = VC @ (SC @ UC^T)
                p1 = wp.tile([N, N], F32, tag="us")
                nc.vector.tensor_scalar_mul(p1[:], uct[:], dc_col[:])
                psa = psp.tile([N, N], F32, tag="psA")
                nc.tensor.matmul(psa[:], vct[:], p1[:], start=True, stop=True)
                nc.vector.tensor_copy(CTf32[:], psa[:])

                # (0.01 B)^T = VB @ (0.01 SB @ UB^T)
                p2 = wp.tile([N, N], F32, tag="us")
                nc.vector.tensor_scalar_mul(p2[:], ubt[:], sb01[:])
                psb = psp.tile([N, N], F32, tag="psB")
                nc.tensor.matmul(psb[:], vbt[:], p2[:], start=True, stop=True)
                nc.vector.tensor_copy(BpTm[:], psb[:])
                # untransposed 0.01 B = UB @ (0.01 SB @ VB^T)
                p2b = wp.tile([N, N], F32, tag="us")
                nc.vector.tensor_scalar_mul(p2b[:], vbt[:], sb01[:])
                psb2 = psp.tile([N, N], F32, tag="psB")
                nc.tensor.matmul(psb2[:], ubt[:], p2b[:], start=True, stop=True)
                bp_un = ep.tile([N, N], F32, tag="Bpun")
                nc.vector.tensor_copy(bp_un[:], psb2[:])

                # M = UA @ (sa05 * UA^T) = 0.5*UA SA UA^T (symmetric)
                p3 = wp.tile([N, N], F32, tag="negx")
                nc.vector.tensor_scalar_mul(p3[:], uat[:], sa05[:])
                psm = psp.tile([N, N], F32, tag="psA")
                nc.tensor.matmul(psm[:], uat[:], p3[:], start=True, stop=True)
                # YA = Uy - Uy^T; q2 = -0.005*YA
                uy = wp.tile([N, N], F32, tag="us")
                nc.vector.tensor_mul(uy[:], zt["ZA_Y"][:], masku[:])
                pst2 = psp.tile([N, N], F32, tag="psB")
                nc.tensor.transpose(pst2[:], uy[:], ident[:])
                nc.vector.tensor_scalar_mul(uy[:], uy[:], 0.5 * STEP)
                q2 = wp.tile([N, N], F32, tag="T")
                nc.vector.scalar_tensor_tensor(
                    q2[:], pst2[:], 0.5 * STEP, uy[:], op0=ALU.mult, op1=ALU.subtract
                )
                # (0.01 A)^T = 0.01*M + q2 ; untransposed 0.01 A = 0.01*M - q2
                nc.vector.scalar_tensor_tensor(
                    A01Tm[:], psm[:], STEP, q2[:], op0=ALU.mult, op1=ALU.add
                )
                a01_un = ep.tile([N, N], F32, tag="A01un")
                nc.vector.scalar_tensor_tensor(
                    a01_un[:], psm[:], STEP, q2[:], op0=ALU.mult, op1=ALU.subtract
                )

                # P^T = C^T + (0.01 A)^T C^T ;  Q^T = (0.01 B)^T C^T
                psw = psp.tile([N, N], F32, tag="psA")
                nc.tensor.matmul(psw[:], a01_un[:], CTf32[:], start=True, stop=True)
                nc.vector.scalar_tensor_tensor(
                    PTm[:], psw[:], 1.0, CTf32[:], op0=ALU.mult, op1=ALU.add
                )
                psq = psp.tile([N, N], F32, tag="psB")
                nc.tensor.matmul(psq[:], bp_un[:], CTf32[:], start=True, stop=True)
                nc.vector.tensor_copy(QTm[:], psq[:])

                # r = 0.01 C bx + by
                psr = psp.tile([N, 1], F32, tag="psA", name="psr")
                nc.tensor.matmul(psr[:], CTf32[:], bxp_c[:], start=True, stop=True)
                nc.vector.scalar_tensor_tensor(
                    r_c[:], psr[:], 1.0, by_c[:], op0=ALU.mult, op1=ALU.add
                )

            # ------- recurrence (split-carry) -------
            with (
                tc.tile_pool(name="xbuf", bufs=2) as xbufp,
                tc.tile_pool(name="stage", bufs=2) as stagep,
                tc.tile_pool(name="xr", bufs=3) as xrp,
                tc.tile_pool(name="th", bufs=3) as thp,
                tc.tile_pool(name="psy", bufs=3, space="PSUM") as psyp,
                tc.tile_pool(name="psx", bufs=3, space="PSUM") as psxp,
                tc.tile_pool(name="pstr", bufs=2, space="PSUM") as pstrp,
            ):
                # xbuf column slot for local step s: pairs (i, i+half) are
                # adjacent so the PE transpose reads one contiguous block
                # (walrus: matmul weight APs must have a single free dim).
                def slot(s):
                    return 2 * (s % half) + (s // half)

                # rounded copy of the state for matmul consumption
                xr_prev = xrp.tile([N, BSH], mdt, tag="xr", name="xr_init")
                nc.vector.tensor_copy(xr_prev[:], x0_c[:])
                # th_0 = tanh(C x_0 + by), fp32 matmul (one-time)
                psy0 = psyp.tile([N, BSH], F32, tag="psy", name="psy0")
                nc.tensor.matmul(psy0[:], CTf32[:], x0_c[:], start=True, stop=True)
                th_prev = thp.tile([N, BSH], mdt, tag="th", name="th_init")
                nc.scalar.activation(
                    th_prev[:], psy0[:], ACTF.Tanh, bias=by_c[:], scale=1.0
                )

                xb_prev = None
                for c in range(nchunks):
                    xb = xbufp.tile([N, tc_chunk * BSH], F32, tag="xb")
                    st = stagep.tile([128, half * N], F32, tag="st")
                    if c == 0:
                        nc.vector.tensor_copy(xb[:, 0:BSH], x0_c[:])
                    for s in range(tc_chunk):
                        t = c * tc_chunk + s
                        if t > 0:
                            if s > 0:
                                pxb, ps_ = xb, slot(s - 1)
                            else:
                                pxb, ps_ = xb_prev, slot(tc_chunk - 1)
                            xprev_f32 = pxb[:, ds(ps_ * BSH, BSH)]
                            # th-chain: y_t = P xr + Q th ; th_t = tanh(y+r)
                            psy = psyp.tile([N, BSH], F32, tag="psy")
                            nc.tensor.matmul(
                                psy[:], QTm[:], th_prev[:], start=True, stop=False
                            )
                            nc.tensor.matmul(
                                psy[:], PTm[:], xr_prev[:], start=False, stop=True
                            )
                            # x-chain: psx = 0.01A xr + 0.01B th
                            psx = psxp.tile([N, BSH], F32, tag="psx")
                            nc.tensor.matmul(
                                psx[:], BpTm[:], th_prev[:], start=True, stop=False
                            )
                            nc.tensor.matmul(
                                psx[:], A01Tm[:], xr_prev[:], start=False, stop=True
                            )
                            th_new = thp.tile([N, BSH], mdt, tag="th")
                            nc.scalar.activation(
                                th_new[:], psy[:], ACTF.Tanh, bias=r_c[:], scale=1.0
                            )
                            # chain-critical: rounded next state for the matmuls
                            xr_new = xrp.tile([N, BSH], mdt, tag="xr")
                            nc.vector.scalar_tensor_tensor(
                                xr_new[:], psx[:], bxp_c[:], xprev_f32,
                                op0=ALU.add, op1=ALU.add,
                            )
                            # exact fp32 state carry (also the output value)
                            nc.vector.scalar_tensor_tensor(
                                xb[:, ds(slot(s) * BSH, BSH)], psx[:], bxp_c[:],
                                xprev_f32, op0=ALU.add, op1=ALU.add,
                            )
                            xr_prev = xr_new
                            th_prev = th_new
                        if s >= half:
                            i = s - half
                            # transpose steps (i, i+half): adjacent slots
                            # (2i, 2i+1) -> one contiguous 128-col block.
                            # Two pair-transposes share one psum tile; a
                            # single ACT copy drains both (fewer ACT ops on
                            # the engine the chain-critical tanh runs on).
                            if i % 2 == 0:
                                pstr = pstrp.tile([128, 2 * N], F32, tag="pstr")
                            nc.tensor.transpose(
                                pstr[:, ds((i % 2) * N, N)],
                                xb[:, ds(2 * i * BSH, 2 * BSH)],
                                ident[:],
                            )
                            if i % 2 == 1 or s == tc_chunk - 1:
                                lo = (i - (i % 2)) * N
                                width = (i % 2 + 1) * N
                                nc.scalar.copy(
                                    st[:, ds(lo, width)], pstr[:, 0:width]
                                )
                    for h in range(2):
                        t0 = c * tc_chunk + h * half
                        dram_ap = out[:, t0:t0 + half, :].rearrange(
                            "b i n -> b (i n)"
                        )
                        nc.sync.dma_start(
                            out=dram_ap, in_=st[h * 64:(h + 1) * 64, :]
                        )
                    xb_prev = xb

    nc.compile()
    return nc


_CACHED = {}


def _get_program(tmax=TMAX, tc_chunk=32, mdt=BF16):
    key = (tmax, tc_chunk, str(mdt))
    if key not in _CACHED:
        _CACHED[key] = build_program(tmax, tc_chunk, mdt)
    return _CACHED[key]


def make_in_maps(inputs, tmax=TMAX):
    X0 = np.ascontiguousarray(np.asarray(inputs["X0"], dtype=np.float32))
    base = {
        name: np.ascontiguousarray(np.asarray(inputs[name], dtype=np.float32))
        for name in PARAM_NAMES
    }
    base["bx"] = np.ascontiguousarray(
        np.asarray(inputs["bx"], dtype=np.float32).reshape(N, 1)
    )
    base["by"] = np.ascontiguousarray(
        np.asarray(inputs["by"], dtype=np.float32).reshape(N, 1)
    )
    in_maps = []
    for c in range(NCORES):
        m = dict(base)
        m["x0"] = np.ascontiguousarray(X0[c * BSH:(c + 1) * BSH].T)
        in_maps.append(m)
    return in_maps


def run_spmd(inputs, tmax=TMAX, tc_chunk=32, trace=False, tmpdir=None, mdt=BF16):
    nc = _get_program(tmax, tc_chunk, mdt)
    in_maps = make_in_maps(inputs, tmax)
    res = run_bass_kernel_spmd(
        nc, in_maps, list(range(NCORES)), trace=trace, tmpdir=tmpdir
    )
    outs = [res.results[c]["out"] for c in range(NCORES)]
    full = np.concatenate(outs, axis=0)
    return full, res


def kernel(**inputs):
    full, _ = run_spmd(inputs)
    return full


# revision 15
# speedup vs baseline: 3.1926x; 1.0158x over previous
"""LurieNet-k Trainium2 kernel.

Computes, from the raw parametrization tensors, the matrices
  C = UC @ SC @ VC^T,  B = UB @ SB @ VB^T,
  A = 0.5*UA @ SA @ UA^T + 0.5*YA  (SA = -(alpha_upp*I + GA))
entirely on device (matrix exponentials of skew matrices via
scaling-and-squaring Taylor), then runs the 511-step recurrence
  y  = C x + by
  x' = x + (0.01*A x + 0.01*B tanh(y) + 0.01*bx)
on a (128, 64) state shard per NeuronCore (batch data-parallel over the
8 cores), writing the full (b, t, n) trajectory.

Precision: the system amplifies per-step state perturbations, so the
identity part of the state update is carried in fp32 OUTSIDE the
matmuls (split-carry): the matmuls only apply gains << 1 (C ~ 0.3,
0.01A ~ 0.003, 0.01B ~ 0.003), so their operands can be bf16 without
error blowup (measured ~2e-3 final rel err vs fp32's 6e-6; naive bf16
with A' = I + 0.01A as a matmul diverges to 0.36). bf16 weights get
FWL weight loads, which matters because the kernel is otherwise
LDWEIGHTS-throughput bound.
"""

import sys

for _p in ("/opt/trn_rl_repo",):
    if _p not in sys.path:
        sys.path.insert(0, _p)

import numpy as np

import concourse.bass as bass
import concourse.mybir as mybir
import concourse.tile as tile
from concourse import bacc
from concourse import bass_isa
from concourse.bass import ds
from concourse.bass_utils import run_bass_kernel_spmd
from concourse.masks import make_identity, make_upper_triangular

F32 = mybir.dt.float32
F32R = mybir.dt.float32r
BF16 = mybir.dt.bfloat16
ALU = mybir.AluOpType
ACTF = mybir.ActivationFunctionType
AXIS = mybir.AxisListType

N = 128          # state dim
TMAX = 512       # time steps (including t=0)
BS = 512         # global batch
NCORES = 8
BSH = BS // NCORES   # 64 batch columns per core
STEP = 0.01
KTOP = 4

EXPM_SCAL = 5    # expm scaling: X = S / 2**EXPM_SCAL, then 5 squarings
EXPM_TERMS = 5   # Taylor terms in the Horner evaluation

PARAM_NAMES = [
    "ZA_Y", "ZA_U", "ZA_G", "ZB_U", "ZB_V", "ZB_S", "ZC_U", "ZC_V", "ZC_S",
]


def build_program(tmax=TMAX, tc_chunk=32, mdt=BF16):
    """Build the single-NeuronCore Bass program (run SPMD on all 8 cores).

    mdt: dtype of the recurrence matmul operands (weights, rounded state
    copy, tanh output). The fp32 state carry is exact regardless.
    """
    assert tmax % tc_chunk == 0 and tc_chunk % 2 == 0
    half = tc_chunk // 2
    nchunks = tmax // tc_chunk

    nc = bacc.Bacc(
        "TRN2",
        target_bir_lowering=False,
        debug=False,
        enable_asserts=False,
        num_devices=NCORES,
    )

    x0 = nc.dram_tensor("x0", [N, BSH], F32, kind="ExternalInput")
    zs = {
        name: nc.dram_tensor(name, [N, N], F32, kind="ExternalInput")
        for name in PARAM_NAMES
    }
    bx_d = nc.dram_tensor("bx", [N, 1], F32, kind="ExternalInput")
    by_d = nc.dram_tensor("by", [N, 1], F32, kind="ExternalInput")
    out = nc.dram_tensor("out", [BSH, tmax, N], F32, kind="ExternalOutput")

    with tile.TileContext(nc) as tc:
        with tc.tile_pool(name="const", bufs=1) as constp:
            ident = constp.tile([N, N], F32, tag="ident")
            make_identity(nc, ident[:])
            masku = constp.tile([N, N], F32, tag="masku")
            make_upper_triangular(nc, masku[:], val=1.0, diag=False)
            ident_r32 = constp.tile([N, N], F32R, tag="ident_r32")
            nc.vector.tensor_copy(ident_r32[:], ident[:])

            by_c = constp.tile([N, 1], F32, tag="by")
            nc.sync.dma_start(out=by_c[:], in_=by_d[:])
            bx_c = constp.tile([N, 1], F32, tag="bxraw")
            nc.sync.dma_start(out=bx_c[:], in_=bx_d[:])
            bxp_c = constp.tile([N, 1], F32, tag="bxp")
            nc.vector.tensor_scalar_mul(bxp_c[:], bx_c[:], STEP)
            x0_c = constp.tile([N, BSH], F32, tag="x0c")
            nc.sync.dma_start(out=x0_c[:], in_=x0[:])

            # ------- setup phase: expm's + weight assembly -------
            # Recurrence uses the P-form to split the serial chain in two:
            #   y_t  = P x_{t-1} + Q th_{t-1} + r,  th_t = tanh(y_t)
            #   x_t  = x_{t-1} + (0.01A) x_{t-1} + (0.01B) th_{t-1} + 0.01 bx
            # with P = C + 0.01 C A, Q = 0.01 C B, r = 0.01 C bx + by.
            PTm = constp.tile([N, N], mdt, tag="PTm")      # P^T
            QTm = constp.tile([N, N], mdt, tag="QTm")      # Q^T
            A01Tm = constp.tile([N, N], mdt, tag="A01Tm")  # (0.01 A)^T
            BpTm = constp.tile([N, N], mdt, tag="BpTm")    # (0.01 B)^T
            CTf32 = constp.tile([N, N], F32, tag="CTf32")  # C^T fp32
            r_c = constp.tile([N, 1], F32, tag="rc")       # 0.01 C bx + by

            with (
                tc.tile_pool(name="zbuf", bufs=1) as zp,
                tc.tile_pool(name="work", bufs=2) as wp,
                tc.tile_pool(name="eres", bufs=1) as ep,
                tc.tile_pool(name="small", bufs=1) as sp,
                tc.tile_pool(name="pss", bufs=4, space="PSUM") as psp,
            ):
                zt = {}
                for name in PARAM_NAMES:
                    zt[name] = zp.tile([N, N], F32, tag=name, name=f"z_{name}")
                    nc.sync.dma_start(out=zt[name][:], in_=zs[name][:])

                def expm_transposed(z_tile, tag):
                    """Return expm(skew(Z))^T for the strict-upper skew of Z.

                    Maintains the (T, T^T) pair through Horner + squaring so
                    no PE transposes are needed: with negX = X^T = -X,
                      X @ T     = matmul(lhsT=negX, rhs=T)
                      T^T @ X^T = matmul(lhsT=T,    rhs=negX)
                    """
                    scal = 1.0 / (2.0 ** EXPM_SCAL)
                    us = wp.tile([N, N], F32R, tag="us_r", name="us_r")
                    # us = (Z * scal) * mask  (strict upper part, prescaled)
                    nc.vector.scalar_tensor_tensor(
                        us[:], z_tile[:], scal, masku[:], op0=ALU.mult, op1=ALU.mult
                    )
                    pst = psp.tile([N, N], F32R, tag="psA", name="pst_r")
                    nc.tensor.transpose(pst[:], us[:], ident_r32[:])
                    negx = wp.tile([N, N], F32R, tag="negx")
                    # negX = us^T - us  ( = X^T = -X for X = us - us^T )
                    nc.vector.scalar_tensor_tensor(
                        negx[:], pst[:], 1.0, us[:], op0=ALU.mult, op1=ALU.subtract
                    )

                    t_cur, tt_cur = ident_r32, ident_r32
                    for j in range(EXPM_TERMS, 0, -1):
                        psa = psp.tile([N, N], F32, tag="psA")
                        psb = psp.tile([N, N], F32, tag="psB")
                        nc.tensor.matmul(
                            psa[:], negx[:], t_cur[:], start=True, stop=True
                        )
                        nc.tensor.matmul(
                            psb[:], t_cur[:], negx[:], start=True, stop=True
                        )
                        t_new = wp.tile([N, N], F32R, tag="T")
                        tt_new = wp.tile([N, N], F32R, tag="TT")
                        nc.vector.scalar_tensor_tensor(
                            t_new[:], psa[:], 1.0 / j, ident_r32[:],
                            op0=ALU.mult, op1=ALU.add,
                        )
                        nc.vector.scalar_tensor_tensor(
                            tt_new[:], psb[:], 1.0 / j, ident_r32[:],
                            op0=ALU.mult, op1=ALU.add,
                        )
                        t_cur, tt_cur = t_new, tt_new
                    for _ in range(EXPM_SCAL):
                        psa = psp.tile([N, N], F32, tag="psA")
                        psb = psp.tile([N, N], F32, tag="psB")
                        nc.tensor.matmul(
                            psa[:], tt_cur[:], t_cur[:], start=True, stop=True
                        )
                        nc.tensor.matmul(
                            psb[:], t_cur[:], tt_cur[:], start=True, stop=True
                        )
                        t_new = wp.tile([N, N], F32R, tag="T")
                        tt_new = wp.tile([N, N], F32R, tag="TT")
                        nc.vector.tensor_copy(t_new[:], psa[:])
                        nc.vector.tensor_copy(tt_new[:], psb[:])
                        t_cur, tt_cur = t_new, tt_new
                    res = ep.tile([N, N], F32, tag=tag)
                    nc.vector.tensor_copy(res[:], tt_cur[:])
                    return res

                uct = expm_transposed(zt["ZC_U"], "UCT")   # UC^T
                vct = expm_transposed(zt["ZC_V"], "VCT")   # VC^T
                ubt = expm_transposed(zt["ZB_U"], "UBT")   # UB^T
                vbt = expm_transposed(zt["ZB_V"], "VBT")   # VB^T
                uat = expm_transposed(zt["ZA_U"], "UAT")   # UA^T

                def absdiag_col(z_tile, tag):
                    tmp = wp.tile([N, N], F32, tag="us")
                    nc.vector.tensor_mul(tmp[:], z_tile[:], ident[:])
                    col = sp.tile([N, 1], F32, tag=tag, name=f"col_{tag}")
                    nc.vector.tensor_reduce(
                        col[:], tmp[:], AXIS.X, ALU.add, apply_absolute_value=True
                    )
                    return col

                dc_col = absdiag_col(zt["ZC_S"], "dc")   # |diag(ZC_S)|
                db_col = absdiag_col(zt["ZB_S"], "db")   # |diag(ZB_S)|
                ga_col = absdiag_col(zt["ZA_G"], "ga")   # |diag(ZA_G)|

                # top-4: alpha = sqrt(sum_i (b_i c_i)^2), b/c sorted desc.
                bwork = sp.tile([N, 1], F32, tag="bwork")
                cwork = sp.tile([N, 1], F32, tag="cwork")
                nc.vector.tensor_copy(bwork[:], db_col[:])
                nc.vector.tensor_copy(cwork[:], dc_col[:])
                acc = sp.tile([N, 1], F32, tag="acc")
                nc.vector.memset(acc[:], 0.0)
                bmax = sp.tile([N, 1], F32, tag="bmax")
                cmax = sp.tile([N, 1], F32, tag="cmax")
                prod = sp.tile([N, 1], F32, tag="prod")
                gmask = sp.tile([N, 1], F32, tag="gmask")
                tdrop = sp.tile([N, 1], F32, tag="tdrop")
                for i in range(KTOP):
                    nc.gpsimd.partition_all_reduce(
                        bmax[:], bwork[:], N, bass_isa.ReduceOp.max
                    )
                    nc.gpsimd.partition_all_reduce(
                        cmax[:], cwork[:], N, bass_isa.ReduceOp.max
                    )
                    nc.vector.tensor_mul(prod[:], bmax[:], cmax[:])
                    nc.vector.tensor_mul(prod[:], prod[:], prod[:])
                    nc.vector.tensor_add(acc[:], acc[:], prod[:])
                    if i < KTOP - 1:
                        # zero out the extracted max (values all > 0)
                        nc.vector.tensor_single_scalar(
                            gmask[:], bwork[:], bmax[:], ALU.is_ge
                        )
                        nc.vector.tensor_mul(tdrop[:], bwork[:], gmask[:])
                        nc.vector.tensor_sub(bwork[:], bwork[:], tdrop[:])
                        nc.vector.tensor_single_scalar(
                            gmask[:], cwork[:], cmax[:], ALU.is_ge
                        )
                        nc.vector.tensor_mul(tdrop[:], cwork[:], gmask[:])
                        nc.vector.tensor_sub(cwork[:], cwork[:], tdrop[:])
                alpha = sp.tile([N, 1], F32, tag="alpha")
                nc.scalar.activation(alpha[:], acc[:], ACTF.Sqrt)

                # sa05 = -0.5*(alpha + gA)  (per-partition row scale of UA^T)
                sa05 = sp.tile([N, 1], F32, tag="sa05")
                nc.vector.tensor_scalar(
                    sa05[:], ga_col[:], alpha[:], -0.5, op0=ALU.add, op1=ALU.mult
                )
                sb01 = sp.tile([N, 1], F32, tag="sb01")
                nc.vector.tensor_scalar_mul(sb01[:], db_col[:], STEP)

                # C^T = VC @ (SC @ UC^T)
                p1 = wp.tile([N, N], F32, tag="us")
                nc.vector.tensor_scalar_mul(p1[:], uct[:], dc_col[:])
                psa = psp.tile([N, N], F32, tag="psA")
                nc.tensor.matmul(psa[:], vct[:], p1[:], start=True, stop=True)
                nc.vector.tensor_copy(CTf32[:], psa[:])

                # (0.01 B)^T = VB @ (0.01 SB @ UB^T)
                p2 = wp.tile([N, N], F32, tag="us")
                nc.vector.tensor_scalar_mul(p2[:], ubt[:], sb01[:])
                psb = psp.tile([N, N], F32, tag="psB")
                nc.tensor.matmul(psb[:], vbt[:], p2[:], start=True, stop=True)
                nc.vector.tensor_copy(BpTm[:], psb[:])
                # untransposed 0.01 B = UB @ (0.01 SB @ VB^T)
                p2b = wp.tile([N, N], F32, tag="us")
                nc.vector.tensor_scalar_mul(p2b[:], vbt[:], sb01[:])
                psb2 = psp.tile([N, N], F32, tag="psB")
                nc.tensor.matmul(psb2[:], ubt[:], p2b[:], start=True, stop=True)
                bp_un = ep.tile([N, N], F32, tag="Bpun")
                nc.vector.tensor_copy(bp_un[:], psb2[:])

                # M = UA @ (sa05 * UA^T) = 0.5*UA SA UA^T (symmetric)
                p3 = wp.tile([N, N], F32, tag="negx")
                nc.vector.tensor_scalar_mul(p3[:], uat[:], sa05[:])
                psm = psp.tile([N, N], F32, tag="psA")
                nc.tensor.matmul(psm[:], uat[:], p3[:], start=True, stop=True)
                # YA = Uy - Uy^T; q2 = -0.005*YA
                uy = wp.tile([N, N], F32, tag="us")
                nc.vector.tensor_mul(uy[:], zt["ZA_Y"][:], masku[:])
                pst2 = psp.tile([N, N], F32, tag="psB")
                nc.tensor.transpose(pst2[:], uy[:], ident[:])
                nc.vector.tensor_scalar_mul(uy[:], uy[:], 0.5 * STEP)
                q2 = wp.tile([N, N], F32, tag="T")
                nc.vector.scalar_tensor_tensor(
                    q2[:], pst2[:], 0.5 * STEP, uy[:], op0=ALU.mult, op1=ALU.subtract
                )
                # (0.01 A)^T = 0.01*M + q2 ; untransposed 0.01 A = 0.01*M - q2
                nc.vector.scalar_tensor_tensor(
                    A01Tm[:], psm[:], STEP, q2[:], op0=ALU.mult, op1=ALU.add
                )
                a01_un = ep.tile([N, N], F32, tag="A01un")
                nc.vector.scalar_tensor_tensor(
                    a01_un[:], psm[:], STEP, q2[:], op0=ALU.mult, op1=ALU.subtract
                )

                # P^T = C^T + (0.01 A)^T C^T ;  Q^T = (0.01 B)^T C^T
                psw = psp.tile([N, N], F32, tag="psA")
                nc.tensor.matmul(psw[:], a01_un[:], CTf32[:], start=True, stop=True)
                nc.vector.scalar_tensor_tensor(
                    PTm[:], psw[:], 1.0, CTf32[:], op0=ALU.mult, op1=ALU.add
                )
                psq = psp.tile([N, N], F32, tag="psB")
                nc.tensor.matmul(psq[:], bp_un[:], CTf32[:], start=True, stop=True)
                nc.vector.tensor_copy(QTm[:], psq[:])

                # r = 0.01 C bx + by
                psr = psp.tile([N, 1], F32, tag="psA", name="psr")
                nc.tensor.matmul(psr[:], CTf32[:], bxp_c[:], start=True, stop=True)
                nc.vector.scalar_tensor_tensor(
                    r_c[:], psr[:], 1.0, by_c[:], op0=ALU.mult, op1=ALU.add
                )

            # ------- recurrence (split-carry) -------
            with (
                tc.tile_pool(name="xbuf", bufs=2) as xbufp,
                tc.tile_pool(name="stage", bufs=2) as stagep,
                tc.tile_pool(name="xr", bufs=3) as xrp,
                tc.tile_pool(name="th", bufs=3) as thp,
                tc.tile_pool(name="psy", bufs=3, space="PSUM") as psyp,
                tc.tile_pool(name="psx", bufs=3, space="PSUM") as psxp,
                tc.tile_pool(name="pstr", bufs=2, space="PSUM") as pstrp,
            ):
                # xbuf column slot for local step s: pairs (i, i+half) are
                # adjacent so the PE transpose reads one contiguous block
                # (walrus: matmul weight APs must have a single free dim).
                def slot(s):
                    return 2 * (s % half) + (s // half)

                # rounded copy of the state for matmul consumption
                xr_prev = xrp.tile([N, BSH], mdt, tag="xr", name="xr_init")
                nc.vector.tensor_copy(xr_prev[:], x0_c[:])
                # th_0 = tanh(C x_0 + by), fp32 matmul (one-time)
                psy0 = psyp.tile([N, BSH], F32, tag="psy", name="psy0")
                nc.tensor.matmul(psy0[:], CTf32[:], x0_c[:], start=True, stop=True)
                th_prev = thp.tile([N, BSH], mdt, tag="th", name="th_init")
                nc.scalar.activation(
                    th_prev[:], psy0[:], ACTF.Tanh, bias=by_c[:], scale=1.0
                )

                xb_prev = None
                for c in range(nchunks):
                    xb = xbufp.tile([N, tc_chunk * BSH], F32, tag="xb")
                    st = stagep.tile([128, half * N], F32, tag="st")
                    if c == 0:
                        nc.vector.tensor_copy(xb[:, 0:BSH], x0_c[:])
                    for s in range(tc_chunk):
                        t = c * tc_chunk + s
                        if t > 0:
                            if s > 0:
                                pxb, ps_ = xb, slot(s - 1)
                            else:
                                pxb, ps_ = xb_prev, slot(tc_chunk - 1)
                            xprev_f32 = pxb[:, ds(ps_ * BSH, BSH)]
                            # th-chain: y_t = P xr + Q th ; th_t = tanh(y+r)
                            psy = psyp.tile([N, BSH], F32, tag="psy")
                            nc.tensor.matmul(
                                psy[:], QTm[:], th_prev[:], start=True, stop=False
                            )
                            nc.tensor.matmul(
                                psy[:], PTm[:], xr_prev[:], start=False, stop=True
                            )
                            # x-chain: psx = 0.01A xr + 0.01B th
                            psx = psxp.tile([N, BSH], F32, tag="psx")
                            nc.tensor.matmul(
                                psx[:], BpTm[:], th_prev[:], start=True, stop=False
                            )
                            nc.tensor.matmul(
                                psx[:], A01Tm[:], xr_prev[:], start=False, stop=True
                            )
                            th_new = thp.tile([N, BSH], mdt, tag="th")
                            nc.scalar.activation(
                                th_new[:], psy[:], ACTF.Tanh, bias=r_c[:], scale=1.0
                            )
                            # chain-critical: rounded next state for the matmuls
                            xr_new = xrp.tile([N, BSH], mdt, tag="xr")
                            nc.vector.scalar_tensor_tensor(
                                xr_new[:], psx[:], bxp_c[:], xprev_f32,
                                op0=ALU.add, op1=ALU.add,
                            )
                            # exact fp32 state carry (also the output value)
                            nc.vector.scalar_tensor_tensor(
                                xb[:, ds(slot(s) * BSH, BSH)], psx[:], bxp_c[:],
                                xprev_f32, op0=ALU.add, op1=ALU.add,
                            )
                            xr_prev = xr_new
                            th_prev = th_new
                        if s >= half:
                            i = s - half
                            # transpose steps (i, i+half): adjacent slots
                            # (2i, 2i+1) -> one contiguous 128-col block.
                            # Two pair-transposes share one psum tile; a
                            # single ACT copy drains both (fewer ACT ops on
                            # the engine the chain-critical tanh runs on).
                            if i % 2 == 0:
                                pstr = pstrp.tile([128, 2 * N], F32, tag="pstr")
                            nc.tensor.transpose(
                                pstr[:, ds((i % 2) * N, N)],
                                xb[:, ds(2 * i * BSH, 2 * BSH)],
                                ident[:],
                            )
                            if i % 2 == 1 or s == tc_chunk - 1:
                                lo = (i - (i % 2)) * N
                                width = (i % 2 + 1) * N
                                nc.scalar.copy(
                                    st[:, ds(lo, width)], pstr[:, 0:width]
                                )
                    qn = max(half // 8, 1)
                    for h in range(2):
                        for q0 in range(0, half, qn):
                            t0 = c * tc_chunk + h * half + q0
                            dram_ap = out[:, t0:t0 + qn, :].rearrange(
                                "b i n -> b (i n)"
                            )
                            nc.sync.dma_start(
                                out=dram_ap,
                                in_=st[h * 64:(h + 1) * 64, ds(q0 * N, qn * N)],
                            )
                    xb_prev = xb

    nc.compile()
    return nc


_CACHED = {}


def _get_program(tmax=TMAX, tc_chunk=32, mdt=BF16):
    key = (tmax, tc_chunk, str(mdt))
    if key not in _CACHED:
        _CACHED[key] = build_program(tmax, tc_chunk, mdt)
    return _CACHED[key]


def make_in_maps(inputs, tmax=TMAX):
    X0 = np.ascontiguousarray(np.asarray(inputs["X0"], dtype=np.float32))
    base = {
        name: np.ascontiguousarray(np.asarray(inputs[name], dtype=np.float32))
        for name in PARAM_NAMES
    }
    base["bx"] = np.ascontiguousarray(
        np.asarray(inputs["bx"], dtype=np.float32).reshape(N, 1)
    )
    base["by"] = np.ascontiguousarray(
        np.asarray(inputs["by"], dtype=np.float32).reshape(N, 1)
    )
    in_maps = []
    for c in range(NCORES):
        m = dict(base)
        m["x0"] = np.ascontiguousarray(X0[c * BSH:(c + 1) * BSH].T)
        in_maps.append(m)
    return in_maps


def run_spmd(inputs, tmax=TMAX, tc_chunk=32, trace=False, tmpdir=None, mdt=BF16):
    nc = _get_program(tmax, tc_chunk, mdt)
    in_maps = make_in_maps(inputs, tmax)
    res = run_bass_kernel_spmd(
        nc, in_maps, list(range(NCORES)), trace=trace, tmpdir=tmpdir
    )
    outs = [res.results[c]["out"] for c in range(NCORES)]
    full = np.concatenate(outs, axis=0)
    return full, res


def kernel(**inputs):
    full, _ = run_spmd(inputs)
    return full
